# revision 1
# baseline (speedup 1.0000x reference)
"""Trainium2 Bass kernel for InpaintingAttnProcessor (3-branch masked SDPA).

Sharding: heads of the two 8-head SDPA branches are split across the 8
cores (1 head each); the single-head d=640 "entity" branch is sharded over
query rows (each core owns 512 queries and computes its k/v projections
locally). Masks are fused into the score matmul as extra contraction rows
(+/- 2^17 bias), softmax runs without max-subtraction (scores are O(5);
masked lanes underflow to exactly 0). One ReduceScatter combines the
per-head Wo partial products; everything else is local.
"""
import numpy as np
from contextlib import ExitStack

import concourse.bass as bass
import concourse.tile as tile
from concourse import bacc, mybir
from concourse.bass_utils import run_bass_kernel_spmd

S, C, H, D = 4096, 640, 8, 80
NCORES = 8
SL = S // NCORES          # 512 queries per core (ent branch + output slice)
BB = 131072.0             # mask bias magnitude (2^17, exact in bf16)
SCALE_H = 1.0 / np.sqrt(80.0)
SCALE_E = 1.0 / np.sqrt(640.0)
F32 = mybir.dt.float32
BF16 = mybir.dt.bfloat16
I32 = mybir.dt.int32
EXP = mybir.ActivationFunctionType.Exp
COPY = mybir.ActivationFunctionType.Copy
EQ = mybir.AluOpType.is_equal
MULT = mybir.AluOpType.mult
ADD = mybir.AluOpType.add

_cache = {}


def _build():
    nc = bacc.Bacc("TRN2", target_bir_lowering=False, debug=False,
                   num_devices=NCORES)
    d = {}
    d["hT"] = nc.dram_tensor("hT", [C, S], F32, kind="ExternalInput")
    d["hq"] = nc.dram_tensor("hq", [C, SL], F32, kind="ExternalInput")
    d["res"] = nc.dram_tensor("res", [SL, C], F32, kind="ExternalInput")
    for w in ("wq", "wk", "wv", "wqo", "wko", "wvo"):
        d[w] = nc.dram_tensor(w, [C, D], F32, kind="ExternalInput")
    for w in ("wqe", "wke", "wve", "wof"):
        d[w] = nc.dram_tensor(w, [C, C], F32, kind="ExternalInput")
    d["woh"] = nc.dram_tensor("woh", [D, C], F32, kind="ExternalInput")
    d["mrow"] = nc.dram_tensor("mrow", [1, S], I32, kind="ExternalInput")
    d["imrow"] = nc.dram_tensor("imrow", [1, S], I32, kind="ExternalInput")
    d["mq"] = nc.dram_tensor("mq", [1, SL], I32, kind="ExternalInput")
    out_d = nc.dram_tensor("out", [SL, C], F32, kind="ExternalOutput")
    P_dram = nc.dram_tensor("P_part", [S, C], F32)
    Pred_dram = nc.dram_tensor("P_red", [SL, C], F32)

    with tile.TileContext(nc) as tc:
        _body(nc, tc, d, out_d, P_dram, Pred_dram)
    nc.compile()
    return nc


def _body(nc, tc, d, out_d, P_dram, Pred_dram):
    ctx = ExitStack()
    with ctx:
        base = ctx.enter_context(tc.tile_pool(name="base", bufs=1))

        # ---------- load + bf16-convert inputs ----------
        hTb = base.tile([128, 5 * S], BF16, tag="hTb")
        wsb = {}
        for w in ("wq", "wk", "wv", "wqo", "wko", "wvo"):
            wsb[w] = base.tile([128, 5 * D], BF16, tag="w_" + w, name="wsb_" + w)
        woh_sb = base.tile([D, C], BF16, tag="woh")

        with tc.tile_pool(name="stage", bufs=2) as stage:
            for cc in range(5):
                st_t = stage.tile([128, S], F32, tag="stg")
                nc.sync.dma_start(st_t[:], d["hT"].ap()[cc * 128:(cc + 1) * 128, :])
                nc.vector.tensor_copy(hTb[:, cc * S:(cc + 1) * S], st_t[:])
            for w in ("wq", "wk", "wv", "wqo", "wko", "wvo"):
                st_t = stage.tile([128, S], F32, tag="stg")
                for cc in range(5):
                    nc.sync.dma_start(st_t[:, cc * D:(cc + 1) * D],
                                      d[w].ap()[cc * 128:(cc + 1) * 128, :])
                nc.vector.tensor_copy(wsb[w][:], st_t[:, 0:5 * D])
            st_t = stage.tile([128, S], F32, tag="stg")
            nc.sync.dma_start(st_t[0:D, 0:C], d["woh"].ap()[:])
            nc.vector.tensor_copy(woh_sb[:], st_t[0:D, 0:C])

        # ---------- mask-derived bias rows ----------
        augk = base.tile([5, S], BF16, tag="augk")    # [onehot_k; 1]
        augko = base.tile([5, S], BF16, tag="augko")  # [onehot_k*im0_k; 1]
        augq = base.tile([5, S], BF16, tag="augq")    # [B*onehot_q; -B]
        augqe = base.tile([5, SL], BF16, tag="augqe")  # ent q-slice bias rows
        iot4i = base.tile([4, 1], I32, tag="iot4i")
        nc.gpsimd.iota(iot4i[:], [[0, 1]], channel_multiplier=1)
        iot4 = base.tile([4, 1], F32, tag="iot4")
        nc.vector.tensor_copy(iot4[:], iot4i[:])
        with tc.tile_pool(name="maskp", bufs=3) as mp:
            mi = mp.tile([4, S], I32, tag="tmp", name="mi")
            for p in range(4):
                nc.sync.dma_start(mi[p:p + 1, :], d["mrow"].ap()[0:1, :])
            mf = mp.tile([4, S], F32, tag="tmp", name="mf")
            nc.vector.tensor_copy(mf[:], mi[:])
            oh = mp.tile([4, S], F32, tag="oh", name="oh")
            nc.vector.tensor_scalar(oh[:], mf[:], iot4[:], None, op0=EQ)
            imi = mp.tile([4, S], I32, tag="tmp", name="imi")
            for p in range(4):
                nc.sync.dma_start(imi[p:p + 1, :], d["imrow"].ap()[0:1, :])
            imf = mp.tile([4, S], F32, tag="tmp", name="imf")
            nc.vector.tensor_copy(imf[:], imi[:])
            im0 = mp.tile([4, S], F32, tag="tmp", name="im0")
            nc.vector.tensor_scalar(im0[:], imf[:], 0.0, None, op0=EQ)
            nc.vector.memset(augk[:], 1.0)
            nc.vector.tensor_copy(augk[0:4, :], oh[:])
            oh0 = mp.tile([4, S], F32, tag="tmp", name="oh0")
            nc.vector.tensor_mul(oh0[:], oh[:], im0[:])
            nc.vector.memset(augko[:], 1.0)
            nc.vector.tensor_copy(augko[0:4, :], oh0[:])
            nc.vector.memset(augq[:], -BB)
            nc.vector.tensor_scalar(augq[0:4, :], oh[:], BB, None, op0=MULT)
            mqi = mp.tile([4, SL], I32, tag="tmp", name="mqi")
            for p in range(4):
                nc.sync.dma_start(mqi[p:p + 1, :], d["mq"].ap()[0:1, :])
            mqf = mp.tile([4, SL], F32, tag="tmp", name="mqf")
            nc.vector.tensor_copy(mqf[:], mqi[:])
            nc.vector.memset(augqe[:], -BB)
            nc.vector.tensor_scalar(augqe[0:4, :], mqf[:], iot4[:], BB,
                                    op0=EQ, op1=MULT)

        ones_bf = base.tile([128, 1], BF16, tag="ones_bf")
        nc.vector.memset(ones_bf[:], 1.0)
        ones_f = base.tile([1, 128], F32, tag="ones_f")
        nc.vector.memset(ones_f[:], 1.0)

        entout = base.tile([128, 4 * C], F32, tag="entout")
        hsTo = base.tile([D, S], BF16, tag="hsTo")
        hsTu = base.tile([D, S], BF16, tag="hsTu")

        # ================= ENT branch (local, q-slice) =================
        # qeT projection [640, SL] bf16, cc-major over d-chunks
        with tc.tile_pool(name="entp", bufs=1) as ep:
            for w in ("wqe", "wke", "wve", "wof"):
                wsb[w] = ep.tile([128, 5 * C], BF16, tag="w_" + w,
                                 name="wsb_" + w)
            hqb = ep.tile([128, 5 * SL], BF16, tag="hqb")
            with tc.tile_pool(name="stage2", bufs=2) as stage2:
                for w in ("wqe", "wke", "wve", "wof"):
                    for cc in range(5):
                        st2 = stage2.tile([128, C], F32, tag="stg2", name="st2")
                        nc.sync.dma_start(st2[:],
                                          d[w].ap()[cc * 128:(cc + 1) * 128, :])
                        nc.vector.tensor_copy(wsb[w][:, cc * C:(cc + 1) * C],
                                              st2[:])
                for cc in range(5):
                    st2 = stage2.tile([128, C], F32, tag="stg2", name="st2")
                    nc.sync.dma_start(st2[:, 0:SL],
                                      d["hq"].ap()[cc * 128:(cc + 1) * 128, :])
                    nc.vector.tensor_copy(hqb[:, cc * SL:(cc + 1) * SL],
                                          st2[:, 0:SL])
            qeb = ep.tile([128, 5 * SL], BF16, tag="qeb")
            eph = ctx_ent = ExitStack()
            ctx_ent.__enter__()
            eps = ctx_ent.enter_context(
                tc.tile_pool(name="ent_ps", bufs=2, space="PSUM"))
            epsS = ctx_ent.enter_context(
                tc.tile_pool(name="ent_psS", bufs=1, space="PSUM"))
            for dc in range(5):
                pq = eps.tile([128, SL], F32, tag="pqe")
                for cc in range(5):
                    nc.tensor.matmul(
                        pq[:],
                        wsb["wqe"][:, cc * C + dc * 128:cc * C + (dc + 1) * 128],
                        hqb[:, cc * SL:(cc + 1) * SL],
                        start=(cc == 0), stop=(cc == 4))
                nc.scalar.activation(qeb[:, dc * SL:(dc + 1) * SL], pq[:],
                                     COPY, scale=SCALE_E)
            # phase a: project keT tile-by-tile, scores, exp -> PTe
            PTe = ep.tile([128, 32 * SL], BF16, tag="PTe")
            with tc.tile_pool(name="kep", bufs=3) as kep, \
                 tc.tile_pool(name="ke_ps", bufs=2, space="PSUM") as keps:
                for g in range(8):
                    pse = epsS.tile([128, 4 * SL], F32, tag="pse")
                    for j in range(4):
                        kt = g * 4 + j
                        k0 = kt * 128
                        ke = kep.tile([128, C], BF16, tag="ke")
                        for dc in range(5):
                            pk = keps.tile([128, 128], F32, tag="pk")
                            for cc in range(5):
                                nc.tensor.matmul(
                                    pk[:],
                                    wsb["wke"][:, cc * C + dc * 128:cc * C + (dc + 1) * 128],
                                    hTb[:, cc * S + k0:cc * S + k0 + 128],
                                    start=(cc == 0), stop=(cc == 4))
                            nc.vector.tensor_copy(ke[:, dc * 128:(dc + 1) * 128], pk[:])
                        for dc in range(5):
                            nc.tensor.matmul(
                                pse[:, j * SL:(j + 1) * SL],
                                ke[:, dc * 128:(dc + 1) * 128],
                                qeb[:, dc * SL:(dc + 1) * SL],
                                start=(dc == 0), stop=False)
                        nc.tensor.matmul(pse[:, j * SL:(j + 1) * SL],
                                         augk[:, k0:k0 + 128], augqe[:],
                                         start=False, stop=True)
                    nc.scalar.activation(PTe[:, g * 4 * SL:(g + 1) * 4 * SL],
                                         pse[:], EXP)
            ctx_ent.__exit__(None, None, None)
            # phase b: v projection + AV accumulation (PSUM-resident)
            with tc.tile_pool(name="vkp", bufs=2) as vkp, \
                 tc.tile_pool(name="av_ps", bufs=1, space="PSUM") as avps, \
                 tc.tile_pool(name="vv_ps", bufs=1, space="PSUM") as vvps:
                pave = [avps.tile([128, SL], F32, tag=f"av{dc}", name=f"pave{dc}")
                        for dc in range(5)]
                pden = avps.tile([1, SL], F32, tag="avden")
                for kt in range(32):
                    k0 = kt * 128
                    pv = vvps.tile([128, C], F32, tag="pv")
                    for (o0, w) in ((0, 512), (512, 128)):
                        for cc in range(5):
                            nc.tensor.matmul(
                                pv[:, o0:o0 + w],
                                hTb[:, cc * S + k0:cc * S + k0 + 128],
                                wsb["wve"][:, cc * C + o0:cc * C + o0 + w],
                                start=(cc == 0), stop=(cc == 4))
                    vk = vkp.tile([128, C], BF16, tag="vk")
                    nc.vector.tensor_copy(vk[:], pv[:])
                    for dc in range(5):
                        nc.tensor.matmul(pave[dc][:],
                                         vk[:, dc * 128:(dc + 1) * 128],
                                         PTe[:, kt * SL:(kt + 1) * SL],
                                         start=(kt == 0), stop=(kt == 31))
                    nc.tensor.matmul(pden[:], ones_bf[:],
                                     PTe[:, kt * SL:(kt + 1) * SL],
                                     start=(kt == 0), stop=(kt == 31))
                # evict + normalize via Wo projection with per-partition scale
                oTe = ep.tile([128, 5 * SL], BF16, tag="oTe")
                for dc in range(5):
                    nc.scalar.activation(oTe[:, dc * SL:(dc + 1) * SL],
                                         pave[dc][:], COPY)
                den = ep.tile([1, SL], F32, tag="den")
                nc.vector.tensor_copy(den[:], pden[:])
                rec = ep.tile([1, SL], F32, tag="rec")
                nc.vector.reciprocal(rec[:], den[:])
                recT = base.tile([128, 4], F32, tag="recT")
                for st in range(4):
                    nc.sync.dma_start(recT[:, st:st + 1],
                                      rec[0:1, st * 128:(st + 1) * 128])
            with tc.tile_pool(name="wo_ps", bufs=2, space="PSUM") as wops:
                for st in range(4):
                    pw = wops.tile([128, C], F32, tag="pwoe")
                    for (o0, w) in ((0, 512), (512, 128)):
                        for cc in range(5):
                            nc.tensor.matmul(
                                pw[:, o0:o0 + w],
                                oTe[:, cc * SL + st * 128:cc * SL + (st + 1) * 128],
                                wsb["wof"][:, cc * C + o0:cc * C + o0 + w],
                                start=(cc == 0), stop=(cc == 4))
                    nc.scalar.activation(entout[:, st * C:(st + 1) * C], pw[:],
                                         COPY, scale=recT[:, st:st + 1])

        # ================= orig + out branches (1 head each) ============
        with tc.tile_pool(name="brp", bufs=1) as bp:
            ctx_pj = ExitStack()
            ctx_pj.__enter__()
            pjps = ctx_pj.enter_context(
                tc.tile_pool(name="pj_ps", bufs=2, space="PSUM"))
            tiles = {}
            for br, (wqn, wkn, wvn, aug) in (("o", ("wq", "wk", "wv", augk)),
                                             ("u", ("wqo", "wko", "wvo", augko))):
                qaug = bp.tile([85, S], BF16, tag=f"qaug{br}")
                kaug = bp.tile([85, S], BF16, tag=f"kaug{br}")
                vsb = bp.tile([128, 32 * 97], BF16, tag=f"vsb{br}")
                nc.gpsimd.memset(vsb[:], 1.0)
                for sc in range(8):
                    s0 = sc * 512
                    pq = pjps.tile([80, 512], F32, tag="pq")
                    for cc in range(5):
                        nc.tensor.matmul(pq[:],
                                         wsb[wqn][:, cc * D:(cc + 1) * D],
                                         hTb[:, cc * S + s0:cc * S + s0 + 512],
                                         start=(cc == 0), stop=(cc == 4))
                    nc.scalar.activation(qaug[0:80, s0:s0 + 512], pq[:],
                                         COPY, scale=SCALE_H)
                    pk = pjps.tile([80, 512], F32, tag="pq")
                    for cc in range(5):
                        nc.tensor.matmul(pk[:],
                                         wsb[wkn][:, cc * D:(cc + 1) * D],
                                         hTb[:, cc * S + s0:cc * S + s0 + 512],
                                         start=(cc == 0), stop=(cc == 4))
                    nc.scalar.activation(kaug[0:80, s0:s0 + 512], pk[:], COPY)
                for st in range(32):
                    pv = pjps.tile([128, 80], F32, tag="pv80")
                    for cc in range(5):
                        nc.tensor.matmul(pv[:],
                                         hTb[:, cc * S + st * 128:cc * S + (st + 1) * 128],
                                         wsb[wvn][:, cc * D:(cc + 1) * D],
                                         start=(cc == 0), stop=(cc == 4))
                    nc.vector.tensor_copy(vsb[:, st * 97:st * 97 + 80], pv[:])
                nc.sync.dma_start(qaug[80:85, :], augq[:])
                nc.sync.dma_start(kaug[80:85, :], aug[:])
                tiles[br] = (qaug, kaug, vsb)

            ctx_pj.__exit__(None, None, None)
            # attention (S^T layout, fused mask bias, no-max softmax)
            with tc.tile_pool(name="ptp", bufs=2) as ptp, \
                 tc.tile_pool(name="otp", bufs=2) as otp, \
                 tc.tile_pool(name="at_ps", bufs=1, space="PSUM") as atps, \
                 tc.tile_pool(name="av2_ps", bufs=2, space="PSUM") as av2ps, \
                 tc.tile_pool(name="b_ps", bufs=2, space="PSUM") as bps:
                for br, hsT in (("o", hsTo), ("u", hsTu)):
                    qaug, kaug, vsb = tiles[br]
                    for qc in range(8):
                        q0 = qc * 512
                        pav = av2ps.tile([97, 512], F32, tag="pav")
                        for g in range(8):
                            ps = atps.tile([128, 2048], F32, tag="psS")
                            for j in range(4):
                                kt = g * 4 + j
                                nc.tensor.matmul(ps[:, j * 512:(j + 1) * 512],
                                                 kaug[:, kt * 128:(kt + 1) * 128],
                                                 qaug[:, q0:q0 + 512],
                                                 start=True, stop=True)
                            pt = ptp.tile([128, 2048], BF16, tag="pt")
                            nc.scalar.activation(pt[:], ps[:], EXP)
                            for j in range(4):
                                kt = g * 4 + j
                                nc.tensor.matmul(pav[:],
                                                 vsb[:, kt * 97:kt * 97 + 97],
                                                 pt[:, j * 512:(j + 1) * 512],
                                                 start=(kt == 0), stop=(kt == 31))
                        ot = otp.tile([80, 512], F32, tag="ot")
                        nc.vector.tensor_copy(ot[:], pav[0:80, :])
                        den1 = otp.tile([1, 512], F32, tag="den1")
                        nc.vector.tensor_copy(den1[:], pav[96:97, :])
                        rec2 = otp.tile([1, 512], F32, tag="rec2")
                        nc.vector.reciprocal(rec2[:], den1[:])
                        pB = bps.tile([80, 512], F32, tag="pB")
                        nc.tensor.matmul(pB[:], ones_f[0:1, 0:80], rec2[:],
                                         start=True, stop=True)
                        nc.vector.tensor_mul(hsT[:, q0:q0 + 512],
                                              ot[:], pB[:])

        # ================= Wo partials + ReduceScatter ==================
        with tc.tile_pool(name="wop", bufs=2) as wop, \
             tc.tile_pool(name="wo2_ps", bufs=2, space="PSUM") as wo2ps:
            for st in range(32):
                pw = wo2ps.tile([128, C], F32, tag="pwo")
                for (o0, w) in ((0, 512), (512, 128)):
                    nc.tensor.matmul(pw[:, o0:o0 + w],
                                     hsTo[:, st * 128:(st + 1) * 128],
                                     woh_sb[:, o0:o0 + w],
                                     start=True, stop=False)
                    nc.tensor.matmul(pw[:, o0:o0 + w],
                                     hsTu[:, st * 128:(st + 1) * 128],
                                     woh_sb[:, o0:o0 + w],
                                     start=False, stop=True)
                pb = wop.tile([128, C], F32, tag="pbuf")
                nc.vector.tensor_copy(pb[:], pw[:])
                nc.sync.dma_start(P_dram.ap()[st * 128:(st + 1) * 128, :], pb[:])
            nc.gpsimd.collective_compute(
                "ReduceScatter", ADD,
                replica_groups=[list(range(NCORES))],
                ins=[P_dram.ap()[:]], outs=[Pred_dram.ap()[:]])

        # ================= final: P_red + ent + residual ================
        with tc.tile_pool(name="finp", bufs=1) as fp:
            pred = fp.tile([128, 4 * C], F32, tag="pred")
            resid = fp.tile([128, 4 * C], F32, tag="resid")
            for st in range(4):
                nc.sync.dma_start(pred[:, st * C:(st + 1) * C],
                                  Pred_dram.ap()[st * 128:(st + 1) * 128, :])
                nc.sync.dma_start(resid[:, st * C:(st + 1) * C],
                                  d["res"].ap()[st * 128:(st + 1) * 128, :])
            outt = fp.tile([128, 4 * C], F32, tag="outt")
            nc.vector.tensor_add(outt[:], pred[:], entout[:])
            nc.vector.tensor_add(outt[:], outt[:], resid[:])
            for st in range(4):
                nc.sync.dma_start(out_d.ap()[st * 128:(st + 1) * 128, :],
                                  outt[:, st * C:(st + 1) * C])


def build_in_maps(hidden_states, mask, inpainting_mask, Wq, Wk, Wv,
                  Wq_ent, Wk_ent, Wv_ent, Wq_out, Wk_out, Wv_out, Wo):
    h = np.asarray(hidden_states[0], np.float32)          # [S, C]
    hT = np.ascontiguousarray(h.T)                         # [C, S]
    m2 = np.asarray(mask[0, 0], np.int32)                  # [512, 512]
    im2 = np.asarray(inpainting_mask[0, 0], np.int32)
    mrow = np.ascontiguousarray(m2[::8, ::8]).reshape(1, S)
    imrow = np.ascontiguousarray(im2[::8, ::8]).reshape(1, S)
    WoT = np.ascontiguousarray(np.asarray(Wo, np.float32).T)  # [C, C]

    def wT(W):
        return np.ascontiguousarray(np.asarray(W, np.float32).T)

    in_maps = []
    for i in range(NCORES):
        hd = slice(D * i, D * (i + 1))
        ql = slice(SL * i, SL * (i + 1))
        in_maps.append({
            "hT": hT,
            "hq": np.ascontiguousarray(hT[:, ql]),
            "res": np.ascontiguousarray(h[ql, :]),
            "wq": wT(Wq[hd]), "wk": wT(Wk[hd]), "wv": wT(Wv[hd]),
            "wqo": wT(Wq_out[hd]), "wko": wT(Wk_out[hd]), "wvo": wT(Wv_out[hd]),
            "wqe": wT(Wq_ent), "wke": wT(Wk_ent), "wve": wT(Wv_ent),
            "wof": WoT, "woh": np.ascontiguousarray(WoT[hd, :]),
            "mrow": mrow, "imrow": imrow,
            "mq": np.ascontiguousarray(mrow[:, ql]),
        })
    return in_maps


def kernel(**inputs):
    in_maps = build_in_maps(**inputs)
    if "nc" not in _cache:
        _cache["nc"] = _build()
    res = run_bass_kernel_spmd(_cache["nc"], in_maps, list(range(NCORES)),
                               trace=False)
    out = np.concatenate([res.results[i]["out"] for i in range(NCORES)], axis=0)
    return out.reshape(1, S, C).astype(np.float32)



# revision 17
# speedup vs baseline: 1.7624x; 1.7624x over previous
"""Trainium2 Bass kernel for InpaintingAttnProcessor (3-branch masked SDPA).

Sharding: heads of the two 8-head SDPA branches are split across the 8
cores (1 head each); the single-head d=640 "entity" branch is sharded over
query rows. Entity k/v projections are computed on key slices and
AllGathered (no duplicated full projections). Masks are fused into the
score matmul as extra contraction rows (+/- 2^17 bias); softmax runs
without max-subtraction. Per-head Wo partial products are combined with a
bf16 ReduceScatter that overlaps the entity branch. The attention inner
loops are software-pipelined (double-buffered score PSUM) so the exp
activations overlap the matmuls and the PE stays HAM-warm.
"""
import numpy as np
import ml_dtypes
from contextlib import ExitStack

import concourse.bass as bass
import concourse.tile as tile
from concourse import bacc, mybir
from concourse.bass_utils import run_bass_kernel_spmd

S, C, H, D = 4096, 640, 8, 80
NCORES = 8
SL = S // NCORES          # 512 queries/keys per core
BB = 131072.0             # mask bias magnitude (2^17, exact in bf16)
F32 = mybir.dt.float32
BF16 = mybir.dt.bfloat16
I32 = mybir.dt.int32
EXP = mybir.ActivationFunctionType.Exp
COPY = mybir.ActivationFunctionType.Copy
EQ = mybir.AluOpType.is_equal
MULT = mybir.AluOpType.mult
ADD = mybir.AluOpType.add
BYPASS = mybir.AluOpType.bypass
RG = [list(range(NCORES))]

_cache = {}


def _build():
    nc = bacc.Bacc("TRN2", target_bir_lowering=False, debug=False,
                   num_devices=NCORES)
    d = {}
    d["hT"] = nc.dram_tensor("hT", [C, S], BF16, kind="ExternalInput")
    d["hq"] = nc.dram_tensor("hq", [C, SL], BF16, kind="ExternalInput")
    d["res"] = nc.dram_tensor("res", [SL, C], F32, kind="ExternalInput")
    for w in ("wq", "wk", "wv", "wqo", "wko", "wvo"):
        d[w] = nc.dram_tensor(w, [C, D], BF16, kind="ExternalInput")
    for w in ("wqe", "wke", "wve", "wof"):
        d[w] = nc.dram_tensor(w, [C, C], BF16, kind="ExternalInput")
    d["woh"] = nc.dram_tensor("woh", [D, C], BF16, kind="ExternalInput")
    d["mrow"] = nc.dram_tensor("mrow", [1, S], I32, kind="ExternalInput")
    d["imrow"] = nc.dram_tensor("imrow", [1, S], I32, kind="ExternalInput")
    d["mq"] = nc.dram_tensor("mq", [1, SL], I32, kind="ExternalInput")
    out_d = nc.dram_tensor("out", [SL, C], F32, kind="ExternalOutput")
    ke_in = nc.dram_tensor("KE_in", [C, SL], BF16)
    ke_out = nc.dram_tensor("KE_out", [NCORES * C, SL], BF16,
                            addr_space="Shared")
    ve_in = nc.dram_tensor("VE_in", [SL, C], BF16)
    ve_out = nc.dram_tensor("VE_out", [S, C], BF16, addr_space="Shared")
    p_part = nc.dram_tensor("P_part", [S, C], BF16)
    p_red = nc.dram_tensor("P_red", [SL, C], BF16)

    with tile.TileContext(nc) as tc:
        _body(nc, tc, d, out_d, ke_in, ke_out, ve_in, ve_out, p_part, p_red)
    nc.compile()
    return nc


def _body(nc, tc, d, out_d, ke_in, ke_out, ve_in, ve_out, p_part, p_red):
    ctx = ExitStack()
    with ctx:
        base = ctx.enter_context(tc.tile_pool(name="base", bufs=1))
        # ---- long-lived tiles ----
        wsb = {}
        for w in ("wq", "wk", "wv", "wqo", "wko", "wvo"):
            wsb[w] = base.tile([128, 5 * D], BF16, tag="w_" + w,
                               name="wsb_" + w)
        woh_sb = base.tile([D, C], BF16, tag="woh")
        qeb = base.tile([128, 5 * SL], BF16, tag="qeb")
        entout = base.tile([128, 4 * C], BF16, tag="entout")
        augk = base.tile([5, S], BF16, tag="augk")
        augqe = base.tile([5, SL], BF16, tag="augqe")
        ones80 = base.tile([1, D], BF16, tag="ones80")
        ones128 = base.tile([128, 1], BF16, tag="ones128")
        recT = base.tile([128, 4], F32, tag="recT")
        iot4i = base.tile([4, 1], I32, tag="iot4i")
        iot4 = base.tile([4, 1], F32, tag="iot4")
        nc.vector.memset(ones80[:], 1.0)
        nc.vector.memset(ones128[:], 1.0)
        nc.gpsimd.iota(iot4i[:], [[0, 1]], channel_multiplier=1)
        nc.vector.tensor_copy(iot4[:], iot4i[:])

        # ---- phase A-D pool: attention working set (outlives ent pool,
        # LIFO with base; freed at kernel end) ----
        ap = ctx.enter_context(tc.tile_pool(name="attn", bufs=1))
        qaug = {}
        kaug = {}
        vsb = {}
        for br in ("o", "u"):
            qaug[br] = ap.tile([85, S], BF16, tag="qaug" + br,
                               name="qaug_" + br)
            kaug[br] = ap.tile([85, S], BF16, tag="kaug" + br,
                               name="kaug_" + br)
            vsb[br] = ap.tile([128, 32 * 97], BF16, tag="vsb" + br,
                              name="vsb_" + br)
        hsT = {"o": ap.tile([D, S], BF16, tag="hsTo", name="hsT_o"),
               "u": ap.tile([D, S], BF16, tag="hsTu", name="hsT_u")}

        # ---- phase A/B transient pool: hT + ent weights + aug sources ----
        ctx_w = ExitStack()
        ctx_w.__enter__()
        pw_pool = ctx_w.enter_context(tc.tile_pool(name="pw", bufs=1))
        hT = pw_pool.tile([128, 5 * S], BF16, tag="hT")
        web = {}
        for w in ("wqe", "wke", "wve"):
            web[w] = pw_pool.tile([128, 5 * C], BF16, tag="w_" + w,
                                  name="web_" + w)
        augq = pw_pool.tile([5, S], BF16, tag="augq")
        augko = pw_pool.tile([5, S], BF16, tag="augko")
        hq_sb = pw_pool.tile([128, 5 * SL], BF16, tag="hq")

        # ---- input DMAs ----
        for w in ("wq", "wk", "wv", "wqo", "wko", "wvo"):
            for cc in range(5):
                nc.sync.dma_start(wsb[w][:, cc * D:(cc + 1) * D],
                                  d[w].ap()[cc * 128:(cc + 1) * 128, :])
        nc.scalar.dma_start(woh_sb[:], d["woh"].ap()[:])
        # hT in (cc, sb-pair) chunks, sb-major so projections start early
        for sbp in range(4):
            for cc in range(5):
                eng = nc.sync if (cc % 2 == 0) else nc.scalar
                eng.dma_start(
                    hT[:, cc * S + sbp * 1024:cc * S + (sbp + 1) * 1024],
                    d["hT"].ap()[cc * 128:(cc + 1) * 128,
                                 sbp * 1024:(sbp + 1) * 1024])
        for w in ("wqe", "wke", "wve"):
            for cc in range(5):
                eng = nc.sync if (cc % 2 == 0) else nc.scalar
                eng.dma_start(web[w][:, cc * C:(cc + 1) * C],
                              d[w].ap()[cc * 128:(cc + 1) * 128, :])
        for cc in range(5):
            nc.scalar.dma_start(hq_sb[:, cc * SL:(cc + 1) * SL],
                                d["hq"].ap()[cc * 128:(cc + 1) * 128, :])

        # ---- mask-derived bias rows (bf16 scratch; exact for 0..3) ----
        with tc.tile_pool(name="maskp", bufs=1) as mp:
            mi = mp.tile([4, S], I32, tag="mi")
            for p in range(4):
                nc.sync.dma_start(mi[p:p + 1, :], d["mrow"].ap()[0:1, :])
            mf = mp.tile([4, S], BF16, tag="mf")
            nc.vector.tensor_copy(mf[:], mi[:])
            nc.vector.memset(augk[:], 1.0)
            nc.vector.tensor_scalar(augk[0:4, :], mf[:], iot4[:], None,
                                    op0=EQ)
            nc.vector.memset(augq[:], -BB)
            nc.vector.tensor_scalar(augq[0:4, :], mf[:], iot4[:], BB,
                                    op0=EQ, op1=MULT)
            imi = mp.tile([4, S], I32, tag="mi", name="imi")
            for p in range(4):
                nc.sync.dma_start(imi[p:p + 1, :], d["imrow"].ap()[0:1, :])
            imf = mp.tile([4, S], BF16, tag="mf", name="imf")
            nc.vector.tensor_copy(imf[:], imi[:])
            im0 = mp.tile([4, S], BF16, tag="im0")
            nc.vector.tensor_scalar(im0[:], imf[:], 0.0, None, op0=EQ)
            nc.vector.memset(augko[:], 1.0)
            nc.vector.tensor_mul(augko[0:4, :], augk[0:4, :], im0[:])
            mqi = mp.tile([4, SL], I32, tag="mqi")
            for p in range(4):
                nc.sync.dma_start(mqi[p:p + 1, :], d["mq"].ap()[0:1, :])
            mqf = mp.tile([4, SL], BF16, tag="mqf")
            nc.vector.tensor_copy(mqf[:], mqi[:])
            nc.vector.memset(augqe[:], -BB)
            nc.vector.tensor_scalar(augqe[0:4, :], mqf[:], iot4[:], BB,
                                    op0=EQ, op1=MULT)

        nc.vector.memset(vsb["o"][:], 1.0)
        nc.vector.memset(vsb["u"][:], 1.0)

        # ================= phase A: head projections ====================
        with tc.tile_pool(name="pjp", bufs=4, space="PSUM") as pjp, \
             tc.tile_pool(name="pvp", bufs=2, space="PSUM") as pvp:
            for br, (wqn, wkn, wvn) in (("o", ("wq", "wk", "wv")),
                                        ("u", ("wqo", "wko", "wvo"))):
                for sb in range(8):
                    s0 = sb * 512
                    pq = pjp.tile([80, 512], F32, tag="pj", name="pq")
                    for cc in range(5):
                        nc.tensor.matmul(
                            pq[:], wsb[wqn][:, cc * D:(cc + 1) * D],
                            hT[:, cc * S + s0:cc * S + s0 + 512],
                            start=(cc == 0), stop=(cc == 4))
                    nc.scalar.activation(qaug[br][0:80, s0:s0 + 512], pq[:],
                                         COPY)
                    pk = pjp.tile([80, 512], F32, tag="pj", name="pk")
                    for cc in range(5):
                        nc.tensor.matmul(
                            pk[:], wsb[wkn][:, cc * D:(cc + 1) * D],
                            hT[:, cc * S + s0:cc * S + s0 + 512],
                            start=(cc == 0), stop=(cc == 4))
                    nc.vector.tensor_copy(kaug[br][0:80, s0:s0 + 512], pk[:])
                for g4 in range(8):
                    pv = pvp.tile([128, 320], F32, tag="pv")
                    for j in range(4):
                        kt = g4 * 4 + j
                        for cc in range(5):
                            nc.tensor.matmul(
                                pv[:, j * 80:(j + 1) * 80],
                                hT[:, cc * S + kt * 128:cc * S + (kt + 1) * 128],
                                wsb[wvn][:, cc * D:(cc + 1) * D],
                                start=(cc == 0), stop=(cc == 4))
                    for j in range(4):
                        kt = g4 * 4 + j
                        nc.vector.tensor_copy(
                            vsb[br][:, kt * 97:kt * 97 + 80],
                            pv[:, j * 80:(j + 1) * 80])
                nc.sync.dma_start(qaug[br][80:85, :], augq[:])
            nc.sync.dma_start(kaug["o"][80:85, :], augk[:])
            nc.sync.dma_start(kaug["u"][80:85, :], augko[:])

        # ================= phase B: ent projections + AllGathers ========
        with tc.tile_pool(name="pje", bufs=4, space="PSUM") as pje, \
             tc.tile_pool(name="pkve", bufs=2, space="PSUM") as pkve, \
             tc.tile_pool(name="stg", bufs=4) as stg:
            # q_e for own query slice (hq = own 512 columns of hT)
            for dc in range(5):
                pq = pje.tile([128, 512], F32, tag="pje", name="pqe")
                for cc in range(5):
                    nc.tensor.matmul(
                        pq[:],
                        web["wqe"][:, cc * C + dc * 128:cc * C + (dc + 1) * 128],
                        hq_sb[:, cc * SL:(cc + 1) * SL],
                        start=(cc == 0), stop=(cc == 4))
                nc.scalar.activation(qeb[:, dc * SL:(dc + 1) * SL], pq[:],
                                     COPY)
            # k_e^T for own key slice -> KE_in [C, SL]
            for dc in range(5):
                pk = pje.tile([128, 512], F32, tag="pje", name="pke")
                for cc in range(5):
                    nc.tensor.matmul(
                        pk[:],
                        web["wke"][:, cc * C + dc * 128:cc * C + (dc + 1) * 128],
                        hq_sb[:, cc * SL:(cc + 1) * SL],
                        start=(cc == 0), stop=(cc == 4))
                kev = stg.tile([128, 512], BF16, tag="kev")
                nc.vector.tensor_copy(kev[:], pk[:])
                nc.sync.dma_start(ke_in.ap()[dc * 128:(dc + 1) * 128, :],
                                  kev[:])
            # v_e for own key slice -> VE_in [SL, C]
            for j in range(4):
                pv = pkve.tile([128, 640], F32, tag="pkve", name="pve")
                for (o0, w) in ((0, 512), (512, 128)):
                    for cc in range(5):
                        nc.tensor.matmul(
                            pv[:, o0:o0 + w],
                            hq_sb[:, cc * SL + j * 128:cc * SL + (j + 1) * 128],
                            web["wve"][:, cc * C + o0:cc * C + o0 + w],
                            start=(cc == 0), stop=(cc == 4))
                vev = stg.tile([128, 640], BF16, tag="vev")
                nc.vector.tensor_copy(vev[:], pv[:])
                nc.sync.dma_start(ve_in.ap()[j * 128:(j + 1) * 128, :],
                                  vev[:])
        nc.gpsimd.collective_compute("AllGather", BYPASS, replica_groups=RG,
                                     ins=[ke_in.ap()[:]],
                                     outs=[ke_out.ap()[:]])
        nc.gpsimd.collective_compute("AllGather", BYPASS, replica_groups=RG,
                                     ins=[ve_in.ap()[:]],
                                     outs=[ve_out.ap()[:]])

        ctx_w.__exit__(None, None, None)   # free hT / ent weights / aug srcs

        # ---- ent pool + early loads of gathered ke/ve (overlap phase C) --
        ctx_e = ExitStack()
        ctx_e.__enter__()
        ep = ctx_e.enter_context(tc.tile_pool(name="entp", bufs=1))
        keT = ep.tile([128, 5 * S], BF16, tag="keT")
        veb = ep.tile([128, 32 * C], BF16, tag="veb")
        wof_sb = ep.tile([128, 5 * C], BF16, tag="wof")
        oTe = ep.tile([128, 5 * SL], BF16, tag="oTe")
        for cc in range(5):
            nc.scalar.dma_start(wof_sb[:, cc * C:(cc + 1) * C],
                                d["wof"].ap()[cc * 128:(cc + 1) * 128, :])
        for dc in range(5):
            for r in range(NCORES):
                eng = nc.sync if (r % 2 == 0) else nc.scalar
                eng.dma_start(
                    keT[:, dc * S + r * SL:dc * S + (r + 1) * SL],
                    ke_out.ap()[r * C + dc * 128:r * C + (dc + 1) * 128, :])
        for kt in range(32):
            eng = nc.sync if (kt % 2 == 0) else nc.scalar
            eng.dma_start(veb[:, kt * C:(kt + 1) * C],
                          ve_out.ap()[kt * 128:(kt + 1) * 128, :])

        # ================= phase C: heads attention =====================
        groups = [(g * 3, 3) for g in range(10)] + [(30, 2)]
        with tc.tile_pool(name="psp", bufs=2, space="PSUM") as psp, \
             tc.tile_pool(name="pavp", bufs=1, space="PSUM") as pavp, \
             tc.tile_pool(name="pBp", bufs=1, space="PSUM") as pBp, \
             tc.tile_pool(name="ptp", bufs=3) as ptp, \
             tc.tile_pool(name="otp", bufs=2) as otp:
            for br in ("o", "u"):
                ka, qa, vs = kaug[br], qaug[br], vsb[br]
                for qc in range(8):
                    q0 = qc * 512
                    pav = pavp.tile([97, 512], F32, tag="pav")
                    pend = None

                    def flush(pend):
                        ps, k0g, nk = pend
                        w = nk * 512
                        pt = ptp.tile([128, 1536], BF16, tag="pt")
                        nc.scalar.activation(pt[:, 0:w], ps[:, 0:w], EXP)
                        for j in range(nk):
                            kt = k0g + j
                            nc.tensor.matmul(
                                pav[:], vs[:, kt * 97:kt * 97 + 97],
                                pt[:, j * 512:(j + 1) * 512],
                                start=(kt == 0), stop=(kt == 31))

                    for (k0g, nk) in groups:
                        ps = psp.tile([128, 1536], F32, tag="ps")
                        for j in range(nk):
                            kt = k0g + j
                            nc.tensor.matmul(
                                ps[:, j * 512:(j + 1) * 512],
                                ka[:, kt * 128:(kt + 1) * 128],
                                qa[:, q0:q0 + 512], start=True, stop=True)
                        if pend is not None:
                            flush(pend)
                        pend = (ps, k0g, nk)
                    flush(pend)
                    # normalize -> hsT
                    den = otp.tile([1, 512], F32, tag="den")
                    nc.vector.tensor_copy(den[:], pav[96:97, :])
                    rec = otp.tile([1, 512], F32, tag="rec")
                    nc.vector.reciprocal(rec[:], den[:])
                    recb = otp.tile([1, 512], BF16, tag="recb")
                    nc.vector.tensor_copy(recb[:], rec[:])
                    ot = otp.tile([80, 512], F32, tag="ot")
                    nc.vector.tensor_copy(ot[:], pav[0:80, :])
                    pB = pBp.tile([80, 512], F32, tag="pB")
                    nc.tensor.matmul(pB[:], ones80[0:1, 0:80], recb[:],
                                     start=True, stop=True)
                    nc.vector.tensor_mul(hsT[br][:, q0:q0 + 512], ot[:],
                                         pB[:])

        # ================= phase D: Wo partials + ReduceScatter =========
        with tc.tile_pool(name="wop", bufs=2) as wop, \
             tc.tile_pool(name="wo2_ps", bufs=2, space="PSUM") as wo2ps:
            for st in range(32):
                pw = wo2ps.tile([128, C], F32, tag="pwo")
                for (o0, w) in ((0, 512), (512, 128)):
                    nc.tensor.matmul(pw[:, o0:o0 + w],
                                     hsT["o"][:, st * 128:(st + 1) * 128],
                                     woh_sb[:, o0:o0 + w],
                                     start=True, stop=False)
                    nc.tensor.matmul(pw[:, o0:o0 + w],
                                     hsT["u"][:, st * 128:(st + 1) * 128],
                                     woh_sb[:, o0:o0 + w],
                                     start=False, stop=True)
                pb = wop.tile([128, C], BF16, tag="pbuf")
                nc.vector.tensor_copy(pb[:], pw[:])
                nc.sync.dma_start(p_part.ap()[st * 128:(st + 1) * 128, :],
                                  pb[:])
        nc.gpsimd.collective_compute("ReduceScatter", ADD, replica_groups=RG,
                                     ins=[p_part.ap()[:]],
                                     outs=[p_red.ap()[:]])

        # ================= phase E: ent attention =======================
        with tc.tile_pool(name="psep", bufs=2, space="PSUM") as psep, \
             tc.tile_pool(name="pavep", bufs=1, space="PSUM") as pavep, \
             tc.tile_pool(name="pdenp", bufs=1, space="PSUM") as pdenp, \
             tc.tile_pool(name="ptep", bufs=3) as ptep, \
             tc.tile_pool(name="ote2", bufs=2) as ot2:
            pave = pavep.tile([128, 5 * SL], F32, tag="pave")
            pden = pdenp.tile([1, SL], F32, tag="pden")
            pend = None

            def flush_e(pend):
                ps, kt = pend
                pt = ptep.tile([128, SL], BF16, tag="pte")
                nc.scalar.activation(pt[:], ps[:], EXP)
                for dc in range(5):
                    nc.tensor.matmul(
                        pave[:, dc * SL:(dc + 1) * SL],
                        veb[:, kt * C + dc * 128:kt * C + (dc + 1) * 128],
                        pt[:], start=(kt == 0), stop=(kt == 31))
                nc.tensor.matmul(pden[:], ones128[:], pt[:],
                                 start=(kt == 0), stop=(kt == 31))

            for kt in range(32):
                ps = psep.tile([128, SL], F32, tag="pse")
                for dc in range(5):
                    nc.tensor.matmul(
                        ps[:],
                        keT[:, dc * S + kt * 128:dc * S + (kt + 1) * 128],
                        qeb[:, dc * SL:(dc + 1) * SL],
                        start=(dc == 0), stop=False)
                nc.tensor.matmul(ps[:], augk[:, kt * 128:(kt + 1) * 128],
                                 augqe[:], start=False, stop=True)
                if pend is not None:
                    flush_e(pend)
                pend = (ps, kt)
            flush_e(pend)
            # evict + normalization scale
            for dc in range(5):
                nc.scalar.activation(oTe[:, dc * SL:(dc + 1) * SL],
                                     pave[:, dc * SL:(dc + 1) * SL], COPY)
            den = ot2.tile([1, SL], F32, tag="dene")
            nc.vector.tensor_copy(den[:], pden[:])
            rec = ot2.tile([1, SL], F32, tag="rece")
            nc.vector.reciprocal(rec[:], den[:])
            for st in range(4):
                nc.sync.dma_start(recT[:, st:st + 1],
                                  rec[0:1, st * 128:(st + 1) * 128])
        with tc.tile_pool(name="wo_ps", bufs=2, space="PSUM") as wops:
            for st in range(4):
                pwe = wops.tile([128, C], F32, tag="pwoe")
                for (o0, w) in ((0, 512), (512, 128)):
                    for dc in range(5):
                        nc.tensor.matmul(
                            pwe[:, o0:o0 + w],
                            oTe[:, dc * SL + st * 128:dc * SL + (st + 1) * 128],
                            wof_sb[:, dc * C + o0:dc * C + o0 + w],
                            start=(dc == 0), stop=(dc == 4))
                nc.scalar.activation(entout[:, st * C:(st + 1) * C], pwe[:],
                                     COPY, scale=recT[:, st:st + 1])

        ctx_e.__exit__(None, None, None)

        # ================= phase F: P_red + ent + residual ==============
        with tc.tile_pool(name="finp", bufs=1) as fp:
            pred = fp.tile([128, 4 * C], BF16, tag="pred")
            resid = fp.tile([128, 4 * C], F32, tag="resid")
            for st in range(4):
                nc.sync.dma_start(pred[:, st * C:(st + 1) * C],
                                  p_red.ap()[st * 128:(st + 1) * 128, :])
                nc.scalar.dma_start(resid[:, st * C:(st + 1) * C],
                                    d["res"].ap()[st * 128:(st + 1) * 128, :])
            outt = fp.tile([128, 4 * C], F32, tag="outt")
            nc.vector.tensor_add(outt[:], pred[:], entout[:])
            nc.vector.tensor_add(outt[:], outt[:], resid[:])
            for st in range(4):
                nc.sync.dma_start(out_d.ap()[st * 128:(st + 1) * 128, :],
                                  outt[:, st * C:(st + 1) * C])


def build_in_maps(hidden_states, mask, inpainting_mask, Wq, Wk, Wv,
                  Wq_ent, Wk_ent, Wv_ent, Wq_out, Wk_out, Wv_out, Wo):
    BF = ml_dtypes.bfloat16
    h = np.asarray(hidden_states[0], np.float32)          # [S, C]
    hT = np.ascontiguousarray(h.T)                         # [C, S]
    m2 = np.asarray(mask[0, 0], np.int32)
    im2 = np.asarray(inpainting_mask[0, 0], np.int32)
    mrow = np.ascontiguousarray(m2[::8, ::8]).reshape(1, S)
    imrow = np.ascontiguousarray(im2[::8, ::8]).reshape(1, S)
    WoT = np.ascontiguousarray(np.asarray(Wo, np.float32).T)  # [C, C]
    sq = np.float32(1.0 / np.sqrt(80.0))
    se = np.float32(1.0 / np.sqrt(640.0))

    def wT(W):
        return np.ascontiguousarray(np.asarray(W, np.float32).T)

    wqe_b = wT(np.asarray(Wq_ent) * se).astype(BF)
    wke_b = wT(Wk_ent).astype(BF)
    wve_b = wT(Wv_ent).astype(BF)
    wof_b = WoT.astype(BF)
    hT_b = hT.astype(BF)

    in_maps = []
    for i in range(NCORES):
        hd = slice(D * i, D * (i + 1))
        ql = slice(SL * i, SL * (i + 1))
        in_maps.append({
            "hT": hT_b,
            "hq": np.ascontiguousarray(hT_b[:, ql]),
            "res": np.ascontiguousarray(h[ql, :]),
            "wq": wT(np.asarray(Wq)[hd] * sq).astype(BF),
            "wk": wT(np.asarray(Wk)[hd]).astype(BF),
            "wv": wT(np.asarray(Wv)[hd]).astype(BF),
            "wqo": wT(np.asarray(Wq_out)[hd] * sq).astype(BF),
            "wko": wT(np.asarray(Wk_out)[hd]).astype(BF),
            "wvo": wT(np.asarray(Wv_out)[hd]).astype(BF),
            "wqe": wqe_b, "wke": wke_b, "wve": wve_b, "wof": wof_b,
            "woh": np.ascontiguousarray(WoT[hd, :]).astype(BF),
            "mrow": mrow, "imrow": imrow,
            "mq": np.ascontiguousarray(mrow[:, ql]),
        })
    return in_maps


def kernel(**inputs):
    in_maps = build_in_maps(**inputs)
    if "nc" not in _cache:
        _cache["nc"] = _build()
    res = run_bass_kernel_spmd(_cache["nc"], in_maps, list(range(NCORES)),
                               trace=False)
    out = np.concatenate([res.results[i]["out"] for i in range(NCORES)], axis=0)
    return out.reshape(1, S, C).astype(np.float32)


# revision 19
# speedup vs baseline: 2.0645x; 1.1714x over previous
"""Trainium2 Bass kernel for InpaintingAttnProcessor (3-branch masked SDPA).

Sharding: heads of the two 8-head SDPA branches are split across the 8
cores (1 head each); the single-head d=640 "entity" branch is sharded over
query rows. Entity k/v projections are computed on key slices and
AllGathered (no duplicated full projections). Masks are fused into the
score matmul as extra contraction rows (+/- 2^17 bias); softmax runs
without max-subtraction. Per-head Wo partial products are combined with a
bf16 ReduceScatter that overlaps the entity branch. The attention inner
loops are software-pipelined (double-buffered score PSUM) so the exp
activations overlap the matmuls and the PE stays HAM-warm.
"""
import numpy as np
import ml_dtypes
from contextlib import ExitStack

import concourse.bass as bass
import concourse.tile as tile
from concourse import bacc, mybir
from concourse.bass_utils import run_bass_kernel_spmd

S, C, H, D = 4096, 640, 8, 80
NCORES = 8
SL = S // NCORES          # 512 queries/keys per core
BB = 131072.0             # mask bias magnitude (2^17, exact in bf16)
F32 = mybir.dt.float32
BF16 = mybir.dt.bfloat16
I32 = mybir.dt.int32
EXP = mybir.ActivationFunctionType.Exp
COPY = mybir.ActivationFunctionType.Copy
EQ = mybir.AluOpType.is_equal
MULT = mybir.AluOpType.mult
ADD = mybir.AluOpType.add
BYPASS = mybir.AluOpType.bypass
RG = [list(range(NCORES))]

_cache = {}


def _build():
    nc = bacc.Bacc("TRN2", target_bir_lowering=False, debug=False,
                   num_devices=NCORES)
    d = {}
    d["hT"] = nc.dram_tensor("hT", [C, S], BF16, kind="ExternalInput")
    d["hq"] = nc.dram_tensor("hq", [C, SL], BF16, kind="ExternalInput")
    d["res"] = nc.dram_tensor("res", [SL, C], F32, kind="ExternalInput")
    for w in ("wq", "wk", "wv", "wqo", "wko", "wvo"):
        d[w] = nc.dram_tensor(w, [C, D], BF16, kind="ExternalInput")
    for w in ("wqe", "wke", "wve", "wof"):
        d[w] = nc.dram_tensor(w, [C, C], BF16, kind="ExternalInput")
    d["woh"] = nc.dram_tensor("woh", [D, C], BF16, kind="ExternalInput")
    d["mrow"] = nc.dram_tensor("mrow", [1, S], I32, kind="ExternalInput")
    d["imrow"] = nc.dram_tensor("imrow", [1, S], I32, kind="ExternalInput")
    d["mq"] = nc.dram_tensor("mq", [1, SL], I32, kind="ExternalInput")
    out_d = nc.dram_tensor("out", [SL, C], F32, kind="ExternalOutput")
    ke_in = nc.dram_tensor("KE_in", [C, SL], BF16)
    ke_out = nc.dram_tensor("KE_out", [NCORES * C, SL], BF16,
                            addr_space="Shared")
    ve_in = nc.dram_tensor("VE_in", [SL, C], BF16)
    ve_out = nc.dram_tensor("VE_out", [S, C], BF16, addr_space="Shared")
    p_part = nc.dram_tensor("P_part", [S, C], BF16)
    p_red = nc.dram_tensor("P_red", [SL, C], BF16)

    with tile.TileContext(nc) as tc:
        _body(nc, tc, d, out_d, ke_in, ke_out, ve_in, ve_out, p_part, p_red)
    nc.compile()
    return nc


def _body(nc, tc, d, out_d, ke_in, ke_out, ve_in, ve_out, p_part, p_red):
    ctx = ExitStack()
    with ctx:
        base = ctx.enter_context(tc.tile_pool(name="base", bufs=1))
        # ---- long-lived tiles ----
        wsb = {}
        for w in ("wq", "wk", "wv", "wqo", "wko", "wvo"):
            wsb[w] = base.tile([128, 5 * D], BF16, tag="w_" + w,
                               name="wsb_" + w)
        woh_sb = base.tile([D, C], BF16, tag="woh")
        qeb = base.tile([128, 5 * SL], BF16, tag="qeb")
        entout = base.tile([128, 4 * C], BF16, tag="entout")
        augk = base.tile([5, S], BF16, tag="augk")
        augqe = base.tile([5, SL], BF16, tag="augqe")
        ones80 = base.tile([1, D], BF16, tag="ones80")
        ones128 = base.tile([128, 1], BF16, tag="ones128")
        recT = base.tile([128, 4], F32, tag="recT")
        iot4i = base.tile([4, 1], I32, tag="iot4i")
        iot4 = base.tile([4, 1], F32, tag="iot4")
        nc.vector.memset(ones80[:], 1.0)
        nc.vector.memset(ones128[:], 1.0)
        nc.gpsimd.iota(iot4i[:], [[0, 1]], channel_multiplier=1)
        nc.vector.tensor_copy(iot4[:], iot4i[:])

        # ---- phase A-D pool: attention working set (outlives ent pool,
        # LIFO with base; freed at kernel end) ----
        ap = ctx.enter_context(tc.tile_pool(name="attn", bufs=1))
        qaug = {}
        kaug = {}
        vsb = {}
        for br in ("o", "u"):
            qaug[br] = ap.tile([85, S], BF16, tag="qaug" + br,
                               name="qaug_" + br)
            kaug[br] = ap.tile([85, S], BF16, tag="kaug" + br,
                               name="kaug_" + br)
            vsb[br] = ap.tile([128, 32 * 97], BF16, tag="vsb" + br,
                              name="vsb_" + br)
        hsT = {"o": ap.tile([D, S], BF16, tag="hsTo", name="hsT_o"),
               "u": ap.tile([D, S], BF16, tag="hsTu", name="hsT_u")}

        # ---- phase A/B transient pool: hT + ent weights + aug sources ----
        ctx_w = ExitStack()
        ctx_w.__enter__()
        pw_pool = ctx_w.enter_context(tc.tile_pool(name="pw", bufs=1))
        hT = pw_pool.tile([128, 5 * S], BF16, tag="hT")
        web = {}
        for w in ("wqe", "wke", "wve"):
            web[w] = pw_pool.tile([128, 5 * C], BF16, tag="w_" + w,
                                  name="web_" + w)
        augq = pw_pool.tile([5, S], BF16, tag="augq")
        augko = pw_pool.tile([5, S], BF16, tag="augko")
        hq_sb = pw_pool.tile([128, 5 * SL], BF16, tag="hq")

        # ---- input DMAs ----
        for w in ("wq", "wk", "wv", "wqo", "wko", "wvo"):
            for cc in range(5):
                nc.sync.dma_start(wsb[w][:, cc * D:(cc + 1) * D],
                                  d[w].ap()[cc * 128:(cc + 1) * 128, :])
        nc.scalar.dma_start(woh_sb[:], d["woh"].ap()[:])
        # hT in (cc, sb-pair) chunks, sb-major so projections start early
        for sbp in range(4):
            for cc in range(5):
                eng = nc.sync if (cc % 2 == 0) else nc.scalar
                eng.dma_start(
                    hT[:, cc * S + sbp * 1024:cc * S + (sbp + 1) * 1024],
                    d["hT"].ap()[cc * 128:(cc + 1) * 128,
                                 sbp * 1024:(sbp + 1) * 1024])
        for w in ("wqe", "wke", "wve"):
            for cc in range(5):
                eng = nc.sync if (cc % 2 == 0) else nc.scalar
                eng.dma_start(web[w][:, cc * C:(cc + 1) * C],
                              d[w].ap()[cc * 128:(cc + 1) * 128, :])
        for cc in range(5):
            nc.scalar.dma_start(hq_sb[:, cc * SL:(cc + 1) * SL],
                                d["hq"].ap()[cc * 128:(cc + 1) * 128, :])

        # ---- mask-derived bias rows (bf16 scratch; exact for 0..3) ----
        with tc.tile_pool(name="maskp", bufs=1) as mp:
            mi = mp.tile([4, S], I32, tag="mi")
            for p in range(4):
                nc.sync.dma_start(mi[p:p + 1, :], d["mrow"].ap()[0:1, :])
            mf = mp.tile([4, S], BF16, tag="mf")
            nc.vector.tensor_copy(mf[:], mi[:])
            nc.vector.memset(augk[:], 1.0)
            nc.vector.tensor_scalar(augk[0:4, :], mf[:], iot4[:], None,
                                    op0=EQ)
            nc.vector.memset(augq[:], -BB)
            nc.vector.tensor_scalar(augq[0:4, :], mf[:], iot4[:], BB,
                                    op0=EQ, op1=MULT)
            imi = mp.tile([4, S], I32, tag="mi", name="imi")
            for p in range(4):
                nc.sync.dma_start(imi[p:p + 1, :], d["imrow"].ap()[0:1, :])
            imf = mp.tile([4, S], BF16, tag="mf", name="imf")
            nc.vector.tensor_copy(imf[:], imi[:])
            im0 = mp.tile([4, S], BF16, tag="im0")
            nc.vector.tensor_scalar(im0[:], imf[:], 0.0, None, op0=EQ)
            nc.vector.memset(augko[:], 1.0)
            nc.vector.tensor_mul(augko[0:4, :], augk[0:4, :], im0[:])
            mqi = mp.tile([4, SL], I32, tag="mqi")
            for p in range(4):
                nc.sync.dma_start(mqi[p:p + 1, :], d["mq"].ap()[0:1, :])
            mqf = mp.tile([4, SL], BF16, tag="mqf")
            nc.vector.tensor_copy(mqf[:], mqi[:])
            nc.vector.memset(augqe[:], -BB)
            nc.vector.tensor_scalar(augqe[0:4, :], mqf[:], iot4[:], BB,
                                    op0=EQ, op1=MULT)

        nc.vector.memset(vsb["o"][:], 1.0)
        nc.vector.memset(vsb["u"][:], 1.0)

        # ================= phase A: head projections ====================
        with tc.tile_pool(name="pjp", bufs=4, space="PSUM") as pjp, \
             tc.tile_pool(name="pvp", bufs=2, space="PSUM") as pvp:
            for br, (wqn, wkn, wvn) in (("o", ("wq", "wk", "wv")),
                                        ("u", ("wqo", "wko", "wvo"))):
                for sb in range(8):
                    s0 = sb * 512
                    pq = pjp.tile([80, 512], F32, tag="pj", name="pq")
                    for cc in range(5):
                        nc.tensor.matmul(
                            pq[:], wsb[wqn][:, cc * D:(cc + 1) * D],
                            hT[:, cc * S + s0:cc * S + s0 + 512],
                            start=(cc == 0), stop=(cc == 4))
                    nc.scalar.activation(qaug[br][0:80, s0:s0 + 512], pq[:],
                                         COPY)
                    pk = pjp.tile([80, 512], F32, tag="pj", name="pk")
                    for cc in range(5):
                        nc.tensor.matmul(
                            pk[:], wsb[wkn][:, cc * D:(cc + 1) * D],
                            hT[:, cc * S + s0:cc * S + s0 + 512],
                            start=(cc == 0), stop=(cc == 4))
                    nc.vector.tensor_copy(kaug[br][0:80, s0:s0 + 512], pk[:])
                for g4 in range(8):
                    pv = pvp.tile([128, 320], F32, tag="pv")
                    for j in range(4):
                        kt = g4 * 4 + j
                        for cc in range(5):
                            nc.tensor.matmul(
                                pv[:, j * 80:(j + 1) * 80],
                                hT[:, cc * S + kt * 128:cc * S + (kt + 1) * 128],
                                wsb[wvn][:, cc * D:(cc + 1) * D],
                                start=(cc == 0), stop=(cc == 4))
                    for j in range(4):
                        kt = g4 * 4 + j
                        nc.vector.tensor_copy(
                            vsb[br][:, kt * 97:kt * 97 + 80],
                            pv[:, j * 80:(j + 1) * 80])
                nc.sync.dma_start(qaug[br][80:85, :], augq[:])
            nc.sync.dma_start(kaug["o"][80:85, :], augk[:])
            nc.sync.dma_start(kaug["u"][80:85, :], augko[:])

        # ================= phase B: ent projections + AllGathers ========
        with tc.tile_pool(name="pje", bufs=4, space="PSUM") as pje, \
             tc.tile_pool(name="pkve", bufs=2, space="PSUM") as pkve, \
             tc.tile_pool(name="stg", bufs=4) as stg:
            # q_e for own query slice (hq = own 512 columns of hT)
            for dc in range(5):
                pq = pje.tile([128, 512], F32, tag="pje", name="pqe")
                for cc in range(5):
                    nc.tensor.matmul(
                        pq[:],
                        web["wqe"][:, cc * C + dc * 128:cc * C + (dc + 1) * 128],
                        hq_sb[:, cc * SL:(cc + 1) * SL],
                        start=(cc == 0), stop=(cc == 4))
                nc.scalar.activation(qeb[:, dc * SL:(dc + 1) * SL], pq[:],
                                     COPY)
            # k_e^T for own key slice -> KE_in [C, SL]
            for dc in range(5):
                pk = pje.tile([128, 512], F32, tag="pje", name="pke")
                for cc in range(5):
                    nc.tensor.matmul(
                        pk[:],
                        web["wke"][:, cc * C + dc * 128:cc * C + (dc + 1) * 128],
                        hq_sb[:, cc * SL:(cc + 1) * SL],
                        start=(cc == 0), stop=(cc == 4))
                kev = stg.tile([128, 512], BF16, tag="kev")
                nc.vector.tensor_copy(kev[:], pk[:])
                nc.sync.dma_start(ke_in.ap()[dc * 128:(dc + 1) * 128, :],
                                  kev[:])
            # v_e for own key slice -> VE_in [SL, C]
            for j in range(4):
                pv = pkve.tile([128, 640], F32, tag="pkve", name="pve")
                for (o0, w) in ((0, 512), (512, 128)):
                    for cc in range(5):
                        nc.tensor.matmul(
                            pv[:, o0:o0 + w],
                            hq_sb[:, cc * SL + j * 128:cc * SL + (j + 1) * 128],
                            web["wve"][:, cc * C + o0:cc * C + o0 + w],
                            start=(cc == 0), stop=(cc == 4))
                vev = stg.tile([128, 640], BF16, tag="vev")
                nc.vector.tensor_copy(vev[:], pv[:])
                nc.sync.dma_start(ve_in.ap()[j * 128:(j + 1) * 128, :],
                                  vev[:])
        nc.gpsimd.collective_compute("AllGather", BYPASS, replica_groups=RG,
                                     ins=[ke_in.ap()[:]],
                                     outs=[ke_out.ap()[:]])
        nc.gpsimd.collective_compute("AllGather", BYPASS, replica_groups=RG,
                                     ins=[ve_in.ap()[:]],
                                     outs=[ve_out.ap()[:]])

        ctx_w.__exit__(None, None, None)   # free hT / ent weights / aug srcs

        # ---- ent pool + early loads of gathered ke/ve (overlap phase C) --
        ctx_e = ExitStack()
        ctx_e.__enter__()
        ep = ctx_e.enter_context(tc.tile_pool(name="entp", bufs=1))
        keT = ep.tile([128, 5 * S], BF16, tag="keT")
        veb = ep.tile([128, 32 * C], BF16, tag="veb")
        wof_sb = ep.tile([128, 5 * C], BF16, tag="wof")
        oTe = ep.tile([128, 5 * SL], BF16, tag="oTe")
        # ================= phase C: heads attention =====================
        groups = [(g * 2, 2) for g in range(16)]
        with tc.tile_pool(name="psp", bufs=3, space="PSUM") as psp, \
             tc.tile_pool(name="pavp", bufs=1, space="PSUM") as pavp, \
             tc.tile_pool(name="pBp", bufs=1, space="PSUM") as pBp, \
             tc.tile_pool(name="ptp", bufs=4) as ptp, \
             tc.tile_pool(name="otp", bufs=2) as otp:
            for br in ("o", "u"):
                ka, qa, vs = kaug[br], qaug[br], vsb[br]
                for qc in range(8):
                    q0 = qc * 512
                    pav = pavp.tile([97, 512], F32, tag="pav")
                    pend = None

                    def flush(pend):
                        ps, k0g, nk = pend
                        w = nk * 512
                        pt = ptp.tile([128, 1024], BF16, tag="pt")
                        nc.scalar.activation(pt[:, 0:w], ps[:, 0:w], EXP)
                        for j in range(nk):
                            kt = k0g + j
                            nc.tensor.matmul(
                                pav[:], vs[:, kt * 97:kt * 97 + 97],
                                pt[:, j * 512:(j + 1) * 512],
                                start=(kt == 0), stop=(kt == 31))

                    for (k0g, nk) in groups:
                        ps = psp.tile([128, 1024], F32, tag="ps")
                        for j in range(nk):
                            kt = k0g + j
                            nc.tensor.matmul(
                                ps[:, j * 512:(j + 1) * 512],
                                ka[:, kt * 128:(kt + 1) * 128],
                                qa[:, q0:q0 + 512], start=True, stop=True)
                        if pend is not None:
                            flush(pend)
                        pend = (ps, k0g, nk)
                    flush(pend)
                    # normalize -> hsT
                    den = otp.tile([1, 512], F32, tag="den")
                    nc.vector.tensor_copy(den[:], pav[96:97, :])
                    rec = otp.tile([1, 512], F32, tag="rec")
                    nc.vector.reciprocal(rec[:], den[:])
                    recb = otp.tile([1, 512], BF16, tag="recb")
                    nc.vector.tensor_copy(recb[:], rec[:])
                    ot = otp.tile([80, 512], F32, tag="ot")
                    nc.vector.tensor_copy(ot[:], pav[0:80, :])
                    pB = pBp.tile([80, 512], F32, tag="pB")
                    nc.tensor.matmul(pB[:], ones80[0:1, 0:80], recb[:],
                                     start=True, stop=True)
                    nc.vector.tensor_mul(hsT[br][:, q0:q0 + 512], ot[:],
                                         pB[:])

        for cc in range(5):
            nc.scalar.dma_start(wof_sb[:, cc * C:(cc + 1) * C],
                                d["wof"].ap()[cc * 128:(cc + 1) * 128, :])
        for dc in range(5):
            for r in range(NCORES):
                eng = nc.sync if (r % 2 == 0) else nc.scalar
                eng.dma_start(
                    keT[:, dc * S + r * SL:dc * S + (r + 1) * SL],
                    ke_out.ap()[r * C + dc * 128:r * C + (dc + 1) * 128, :])
        for kt in range(32):
            eng = nc.sync if (kt % 2 == 0) else nc.scalar
            eng.dma_start(veb[:, kt * C:(kt + 1) * C],
                          ve_out.ap()[kt * 128:(kt + 1) * 128, :])

        # ================= phase D: Wo partials + ReduceScatter =========
        with tc.tile_pool(name="wop", bufs=2) as wop, \
             tc.tile_pool(name="wo2_ps", bufs=2, space="PSUM") as wo2ps:
            for st in range(32):
                pw = wo2ps.tile([128, C], F32, tag="pwo")
                for (o0, w) in ((0, 512), (512, 128)):
                    nc.tensor.matmul(pw[:, o0:o0 + w],
                                     hsT["o"][:, st * 128:(st + 1) * 128],
                                     woh_sb[:, o0:o0 + w],
                                     start=True, stop=False)
                    nc.tensor.matmul(pw[:, o0:o0 + w],
                                     hsT["u"][:, st * 128:(st + 1) * 128],
                                     woh_sb[:, o0:o0 + w],
                                     start=False, stop=True)
                pb = wop.tile([128, C], BF16, tag="pbuf")
                nc.vector.tensor_copy(pb[:], pw[:])
                nc.sync.dma_start(p_part.ap()[st * 128:(st + 1) * 128, :],
                                  pb[:])
        nc.gpsimd.collective_compute("ReduceScatter", ADD, replica_groups=RG,
                                     ins=[p_part.ap()[:]],
                                     outs=[p_red.ap()[:]])

        # ================= phase E: ent attention =======================
        with tc.tile_pool(name="psep", bufs=2, space="PSUM") as psep, \
             tc.tile_pool(name="pavep", bufs=1, space="PSUM") as pavep, \
             tc.tile_pool(name="pdenp", bufs=1, space="PSUM") as pdenp, \
             tc.tile_pool(name="ptep", bufs=3) as ptep, \
             tc.tile_pool(name="ote2", bufs=2) as ot2:
            pave = pavep.tile([128, 5 * SL], F32, tag="pave")
            pden = pdenp.tile([1, SL], F32, tag="pden")
            pend = None

            def flush_e(pend):
                ps, kt = pend
                pt = ptep.tile([128, SL], BF16, tag="pte")
                nc.scalar.activation(pt[:], ps[:], EXP)
                for dc in range(5):
                    nc.tensor.matmul(
                        pave[:, dc * SL:(dc + 1) * SL],
                        veb[:, kt * C + dc * 128:kt * C + (dc + 1) * 128],
                        pt[:], start=(kt == 0), stop=(kt == 31))
                nc.tensor.matmul(pden[:], ones128[:], pt[:],
                                 start=(kt == 0), stop=(kt == 31))

            for kt in range(32):
                ps = psep.tile([128, SL], F32, tag="pse")
                for dc in range(5):
                    nc.tensor.matmul(
                        ps[:],
                        keT[:, dc * S + kt * 128:dc * S + (kt + 1) * 128],
                        qeb[:, dc * SL:(dc + 1) * SL],
                        start=(dc == 0), stop=False)
                nc.tensor.matmul(ps[:], augk[:, kt * 128:(kt + 1) * 128],
                                 augqe[:], start=False, stop=True)
                if pend is not None:
                    flush_e(pend)
                pend = (ps, kt)
            flush_e(pend)
            # evict + normalization scale
            for dc in range(5):
                nc.scalar.activation(oTe[:, dc * SL:(dc + 1) * SL],
                                     pave[:, dc * SL:(dc + 1) * SL], COPY)
            den = ot2.tile([1, SL], F32, tag="dene")
            nc.vector.tensor_copy(den[:], pden[:])
            rec = ot2.tile([1, SL], F32, tag="rece")
            nc.vector.reciprocal(rec[:], den[:])
            for st in range(4):
                nc.sync.dma_start(recT[:, st:st + 1],
                                  rec[0:1, st * 128:(st + 1) * 128])
        with tc.tile_pool(name="wo_ps", bufs=2, space="PSUM") as wops:
            for st in range(4):
                pwe = wops.tile([128, C], F32, tag="pwoe")
                for (o0, w) in ((0, 512), (512, 128)):
                    for dc in range(5):
                        nc.tensor.matmul(
                            pwe[:, o0:o0 + w],
                            oTe[:, dc * SL + st * 128:dc * SL + (st + 1) * 128],
                            wof_sb[:, dc * C + o0:dc * C + o0 + w],
                            start=(dc == 0), stop=(dc == 4))
                nc.scalar.activation(entout[:, st * C:(st + 1) * C], pwe[:],
                                     COPY, scale=recT[:, st:st + 1])

        ctx_e.__exit__(None, None, None)

        # ================= phase F: P_red + ent + residual ==============
        with tc.tile_pool(name="finp", bufs=1) as fp:
            pred = fp.tile([128, 4 * C], BF16, tag="pred")
            resid = fp.tile([128, 4 * C], F32, tag="resid")
            for st in range(4):
                nc.sync.dma_start(pred[:, st * C:(st + 1) * C],
                                  p_red.ap()[st * 128:(st + 1) * 128, :])
                nc.scalar.dma_start(resid[:, st * C:(st + 1) * C],
                                    d["res"].ap()[st * 128:(st + 1) * 128, :])
            outt = fp.tile([128, 4 * C], F32, tag="outt")
            nc.vector.tensor_add(outt[:], pred[:], entout[:])
            nc.vector.tensor_add(outt[:], outt[:], resid[:])
            for st in range(4):
                nc.sync.dma_start(out_d.ap()[st * 128:(st + 1) * 128, :],
                                  outt[:, st * C:(st + 1) * C])


def build_in_maps(hidden_states, mask, inpainting_mask, Wq, Wk, Wv,
                  Wq_ent, Wk_ent, Wv_ent, Wq_out, Wk_out, Wv_out, Wo):
    BF = ml_dtypes.bfloat16
    h = np.asarray(hidden_states[0], np.float32)          # [S, C]
    hT = np.ascontiguousarray(h.T)                         # [C, S]
    m2 = np.asarray(mask[0, 0], np.int32)
    im2 = np.asarray(inpainting_mask[0, 0], np.int32)
    mrow = np.ascontiguousarray(m2[::8, ::8]).reshape(1, S)
    imrow = np.ascontiguousarray(im2[::8, ::8]).reshape(1, S)
    WoT = np.ascontiguousarray(np.asarray(Wo, np.float32).T)  # [C, C]
    sq = np.float32(1.0 / np.sqrt(80.0))
    se = np.float32(1.0 / np.sqrt(640.0))

    def wT(W):
        return np.ascontiguousarray(np.asarray(W, np.float32).T)

    wqe_b = wT(np.asarray(Wq_ent) * se).astype(BF)
    wke_b = wT(Wk_ent).astype(BF)
    wve_b = wT(Wv_ent).astype(BF)
    wof_b = WoT.astype(BF)
    hT_b = hT.astype(BF)

    in_maps = []
    for i in range(NCORES):
        hd = slice(D * i, D * (i + 1))
        ql = slice(SL * i, SL * (i + 1))
        in_maps.append({
            "hT": hT_b,
            "hq": np.ascontiguousarray(hT_b[:, ql]),
            "res": np.ascontiguousarray(h[ql, :]),
            "wq": wT(np.asarray(Wq)[hd] * sq).astype(BF),
            "wk": wT(np.asarray(Wk)[hd]).astype(BF),
            "wv": wT(np.asarray(Wv)[hd]).astype(BF),
            "wqo": wT(np.asarray(Wq_out)[hd] * sq).astype(BF),
            "wko": wT(np.asarray(Wk_out)[hd]).astype(BF),
            "wvo": wT(np.asarray(Wv_out)[hd]).astype(BF),
            "wqe": wqe_b, "wke": wke_b, "wve": wve_b, "wof": wof_b,
            "woh": np.ascontiguousarray(WoT[hd, :]).astype(BF),
            "mrow": mrow, "imrow": imrow,
            "mq": np.ascontiguousarray(mrow[:, ql]),
        })
    return in_maps


def kernel(**inputs):
    in_maps = build_in_maps(**inputs)
    if "nc" not in _cache:
        _cache["nc"] = _build()
    res = run_bass_kernel_spmd(_cache["nc"], in_maps, list(range(NCORES)),
                               trace=False)
    out = np.concatenate([res.results[i]["out"] for i in range(NCORES)], axis=0)
    return out.reshape(1, S, C).astype(np.float32)


# revision 20
# speedup vs baseline: 2.1022x; 1.0183x over previous
"""Trainium2 Bass kernel for InpaintingAttnProcessor (3-branch masked SDPA).

Sharding: heads of the two 8-head SDPA branches are split across the 8
cores (1 head each); the single-head d=640 "entity" branch is sharded over
query rows. Entity k/v projections are computed on key slices and
AllGathered (no duplicated full projections). Masks are fused into the
score matmul as extra contraction rows (+/- 2^17 bias); softmax runs
without max-subtraction. Per-head Wo partial products are combined with a
bf16 ReduceScatter that overlaps the entity branch. The attention inner
loops are software-pipelined (double-buffered score PSUM) so the exp
activations overlap the matmuls and the PE stays HAM-warm.
"""
import numpy as np
import ml_dtypes
from contextlib import ExitStack

import concourse.bass as bass
import concourse.tile as tile
from concourse import bacc, mybir
from concourse.bass_utils import run_bass_kernel_spmd

S, C, H, D = 4096, 640, 8, 80
NCORES = 8
SL = S // NCORES          # 512 queries/keys per core
BB = 131072.0             # mask bias magnitude (2^17, exact in bf16)
F32 = mybir.dt.float32
BF16 = mybir.dt.bfloat16
I32 = mybir.dt.int32
EXP = mybir.ActivationFunctionType.Exp
COPY = mybir.ActivationFunctionType.Copy
EQ = mybir.AluOpType.is_equal
MULT = mybir.AluOpType.mult
ADD = mybir.AluOpType.add
BYPASS = mybir.AluOpType.bypass
RG = [list(range(NCORES))]

_cache = {}


def _build():
    nc = bacc.Bacc("TRN2", target_bir_lowering=False, debug=False,
                   num_devices=NCORES)
    d = {}
    d["hT"] = nc.dram_tensor("hT", [C, S], BF16, kind="ExternalInput")
    d["hq"] = nc.dram_tensor("hq", [C, SL], BF16, kind="ExternalInput")
    d["res"] = nc.dram_tensor("res", [SL, C], F32, kind="ExternalInput")
    for w in ("wq", "wk", "wv", "wqo", "wko", "wvo"):
        d[w] = nc.dram_tensor(w, [C, D], BF16, kind="ExternalInput")
    for w in ("wqe", "wke", "wve", "wof"):
        d[w] = nc.dram_tensor(w, [C, C], BF16, kind="ExternalInput")
    d["woh"] = nc.dram_tensor("woh", [D, C], BF16, kind="ExternalInput")
    d["mrow"] = nc.dram_tensor("mrow", [1, S], I32, kind="ExternalInput")
    d["imrow"] = nc.dram_tensor("imrow", [1, S], I32, kind="ExternalInput")
    d["mq"] = nc.dram_tensor("mq", [1, SL], I32, kind="ExternalInput")
    out_d = nc.dram_tensor("out", [SL, C], F32, kind="ExternalOutput")
    ke_in = nc.dram_tensor("KE_in", [C, SL], BF16)
    ke_out = nc.dram_tensor("KE_out", [NCORES * C, SL], BF16,
                            addr_space="Shared")
    ve_in = nc.dram_tensor("VE_in", [SL, C], BF16)
    ve_out = nc.dram_tensor("VE_out", [S, C], BF16, addr_space="Shared")
    p_part = nc.dram_tensor("P_part", [S, C], BF16)
    p_red = nc.dram_tensor("P_red", [SL, C], BF16)

    with tile.TileContext(nc) as tc:
        _body(nc, tc, d, out_d, ke_in, ke_out, ve_in, ve_out, p_part, p_red)
    nc.compile()
    return nc


def _body(nc, tc, d, out_d, ke_in, ke_out, ve_in, ve_out, p_part, p_red):
    ctx = ExitStack()
    with ctx:
        base = ctx.enter_context(tc.tile_pool(name="base", bufs=1))
        # ---- long-lived tiles ----
        wsb = {}
        for w in ("wq", "wk", "wv", "wqo", "wko", "wvo"):
            wsb[w] = base.tile([128, 5 * D], BF16, tag="w_" + w,
                               name="wsb_" + w)
        woh_sb = base.tile([D, C], BF16, tag="woh")
        qeb = base.tile([128, 5 * SL], BF16, tag="qeb")
        entout = base.tile([128, 4 * C], BF16, tag="entout")
        augk = base.tile([5, S], BF16, tag="augk")
        augqe = base.tile([5, SL], BF16, tag="augqe")
        ones80 = base.tile([1, D], BF16, tag="ones80")
        ones128 = base.tile([128, 1], BF16, tag="ones128")
        recT = base.tile([128, 4], F32, tag="recT")
        iot4i = base.tile([4, 1], I32, tag="iot4i")
        iot4 = base.tile([4, 1], F32, tag="iot4")
        nc.vector.memset(ones80[:], 1.0)
        nc.vector.memset(ones128[:], 1.0)
        nc.gpsimd.iota(iot4i[:], [[0, 1]], channel_multiplier=1)
        nc.vector.tensor_copy(iot4[:], iot4i[:])

        # ---- phase A-D pool: attention working set (outlives ent pool,
        # LIFO with base; freed at kernel end) ----
        ap = ctx.enter_context(tc.tile_pool(name="attn", bufs=1))
        qaug = {}
        kaug = {}
        vsb = {}
        for br in ("o", "u"):
            qaug[br] = ap.tile([85, S], BF16, tag="qaug" + br,
                               name="qaug_" + br)
            kaug[br] = ap.tile([85, S], BF16, tag="kaug" + br,
                               name="kaug_" + br)
            vsb[br] = ap.tile([128, 32 * 97], BF16, tag="vsb" + br,
                              name="vsb_" + br)
        hsT = {"o": ap.tile([D, S], BF16, tag="hsTo", name="hsT_o"),
               "u": ap.tile([D, S], BF16, tag="hsTu", name="hsT_u")}

        # ---- phase A/B transient pool: hT + ent weights + aug sources ----
        ctx_w = ExitStack()
        ctx_w.__enter__()
        pw_pool = ctx_w.enter_context(tc.tile_pool(name="pw", bufs=1))
        hT = pw_pool.tile([128, 5 * S], BF16, tag="hT")
        web = {}
        for w in ("wqe", "wke", "wve"):
            web[w] = pw_pool.tile([128, 5 * C], BF16, tag="w_" + w,
                                  name="web_" + w)
        augq = pw_pool.tile([5, S], BF16, tag="augq")
        augko = pw_pool.tile([5, S], BF16, tag="augko")
        hq_sb = pw_pool.tile([128, 5 * SL], BF16, tag="hq")

        # ---- input DMAs ----
        for w in ("wq", "wk", "wv", "wqo", "wko", "wvo"):
            for cc in range(5):
                nc.sync.dma_start(wsb[w][:, cc * D:(cc + 1) * D],
                                  d[w].ap()[cc * 128:(cc + 1) * 128, :])
        nc.scalar.dma_start(woh_sb[:], d["woh"].ap()[:])
        # hT in (cc, sb-pair) chunks, sb-major so projections start early
        for sbp in range(4):
            for cc in range(5):
                eng = nc.sync if (cc % 2 == 0) else nc.scalar
                eng.dma_start(
                    hT[:, cc * S + sbp * 1024:cc * S + (sbp + 1) * 1024],
                    d["hT"].ap()[cc * 128:(cc + 1) * 128,
                                 sbp * 1024:(sbp + 1) * 1024])
        for w in ("wqe", "wke", "wve"):
            for cc in range(5):
                eng = nc.sync if (cc % 2 == 0) else nc.scalar
                eng.dma_start(web[w][:, cc * C:(cc + 1) * C],
                              d[w].ap()[cc * 128:(cc + 1) * 128, :])
        for cc in range(5):
            nc.scalar.dma_start(hq_sb[:, cc * SL:(cc + 1) * SL],
                                d["hq"].ap()[cc * 128:(cc + 1) * 128, :])

        # ---- mask-derived bias rows (bf16 scratch; exact for 0..3) ----
        with tc.tile_pool(name="maskp", bufs=1) as mp:
            mi = mp.tile([4, S], I32, tag="mi")
            for p in range(4):
                nc.sync.dma_start(mi[p:p + 1, :], d["mrow"].ap()[0:1, :])
            mf = mp.tile([4, S], BF16, tag="mf")
            nc.vector.tensor_copy(mf[:], mi[:])
            nc.vector.memset(augk[:], 1.0)
            nc.vector.tensor_scalar(augk[0:4, :], mf[:], iot4[:], None,
                                    op0=EQ)
            nc.vector.memset(augq[:], -BB)
            nc.vector.tensor_scalar(augq[0:4, :], mf[:], iot4[:], BB,
                                    op0=EQ, op1=MULT)
            imi = mp.tile([4, S], I32, tag="mi", name="imi")
            for p in range(4):
                nc.sync.dma_start(imi[p:p + 1, :], d["imrow"].ap()[0:1, :])
            imf = mp.tile([4, S], BF16, tag="mf", name="imf")
            nc.vector.tensor_copy(imf[:], imi[:])
            im0 = mp.tile([4, S], BF16, tag="im0")
            nc.vector.tensor_scalar(im0[:], imf[:], 0.0, None, op0=EQ)
            nc.vector.memset(augko[:], 1.0)
            nc.vector.tensor_mul(augko[0:4, :], augk[0:4, :], im0[:])
            mqi = mp.tile([4, SL], I32, tag="mqi")
            for p in range(4):
                nc.sync.dma_start(mqi[p:p + 1, :], d["mq"].ap()[0:1, :])
            mqf = mp.tile([4, SL], BF16, tag="mqf")
            nc.vector.tensor_copy(mqf[:], mqi[:])
            nc.vector.memset(augqe[:], -BB)
            nc.vector.tensor_scalar(augqe[0:4, :], mqf[:], iot4[:], BB,
                                    op0=EQ, op1=MULT)

        nc.vector.memset(vsb["o"][:], 1.0)
        nc.vector.memset(vsb["u"][:], 1.0)

        # ================= phase A: head projections ====================
        with tc.tile_pool(name="pjp", bufs=4, space="PSUM") as pjp, \
             tc.tile_pool(name="pvp", bufs=2, space="PSUM") as pvp:
            for br, (wqn, wkn, wvn) in (("o", ("wq", "wk", "wv")),
                                        ("u", ("wqo", "wko", "wvo"))):
                for sb in range(8):
                    s0 = sb * 512
                    pq = pjp.tile([80, 512], F32, tag="pj", name="pq")
                    for cc in range(5):
                        nc.tensor.matmul(
                            pq[:], wsb[wqn][:, cc * D:(cc + 1) * D],
                            hT[:, cc * S + s0:cc * S + s0 + 512],
                            start=(cc == 0), stop=(cc == 4))
                    nc.scalar.activation(qaug[br][0:80, s0:s0 + 512], pq[:],
                                         COPY)
                    pk = pjp.tile([80, 512], F32, tag="pj", name="pk")
                    for cc in range(5):
                        nc.tensor.matmul(
                            pk[:], wsb[wkn][:, cc * D:(cc + 1) * D],
                            hT[:, cc * S + s0:cc * S + s0 + 512],
                            start=(cc == 0), stop=(cc == 4))
                    nc.vector.tensor_copy(kaug[br][0:80, s0:s0 + 512], pk[:])
                for g4 in range(8):
                    pv = pvp.tile([128, 320], F32, tag="pv")
                    for j in range(4):
                        kt = g4 * 4 + j
                        for cc in range(5):
                            nc.tensor.matmul(
                                pv[:, j * 80:(j + 1) * 80],
                                hT[:, cc * S + kt * 128:cc * S + (kt + 1) * 128],
                                wsb[wvn][:, cc * D:(cc + 1) * D],
                                start=(cc == 0), stop=(cc == 4))
                    for j in range(4):
                        kt = g4 * 4 + j
                        nc.vector.tensor_copy(
                            vsb[br][:, kt * 97:kt * 97 + 80],
                            pv[:, j * 80:(j + 1) * 80])
                nc.sync.dma_start(qaug[br][80:85, :], augq[:])
            nc.sync.dma_start(kaug["o"][80:85, :], augk[:])
            nc.sync.dma_start(kaug["u"][80:85, :], augko[:])

        # ================= phase B: ent projections + AllGathers ========
        with tc.tile_pool(name="pje", bufs=4, space="PSUM") as pje, \
             tc.tile_pool(name="pkve", bufs=2, space="PSUM") as pkve, \
             tc.tile_pool(name="stg", bufs=4) as stg:
            # q_e for own query slice (hq = own 512 columns of hT)
            for dc in range(5):
                pq = pje.tile([128, 512], F32, tag="pje", name="pqe")
                for cc in range(5):
                    nc.tensor.matmul(
                        pq[:],
                        web["wqe"][:, cc * C + dc * 128:cc * C + (dc + 1) * 128],
                        hq_sb[:, cc * SL:(cc + 1) * SL],
                        start=(cc == 0), stop=(cc == 4))
                nc.scalar.activation(qeb[:, dc * SL:(dc + 1) * SL], pq[:],
                                     COPY)
            # k_e^T for own key slice -> KE_in [C, SL]
            for dc in range(5):
                pk = pje.tile([128, 512], F32, tag="pje", name="pke")
                for cc in range(5):
                    nc.tensor.matmul(
                        pk[:],
                        web["wke"][:, cc * C + dc * 128:cc * C + (dc + 1) * 128],
                        hq_sb[:, cc * SL:(cc + 1) * SL],
                        start=(cc == 0), stop=(cc == 4))
                kev = stg.tile([128, 512], BF16, tag="kev")
                nc.vector.tensor_copy(kev[:], pk[:])
                nc.sync.dma_start(ke_in.ap()[dc * 128:(dc + 1) * 128, :],
                                  kev[:])
            # v_e for own key slice -> VE_in [SL, C]
            for j in range(4):
                pv = pkve.tile([128, 640], F32, tag="pkve", name="pve")
                for (o0, w) in ((0, 512), (512, 128)):
                    for cc in range(5):
                        nc.tensor.matmul(
                            pv[:, o0:o0 + w],
                            hq_sb[:, cc * SL + j * 128:cc * SL + (j + 1) * 128],
                            web["wve"][:, cc * C + o0:cc * C + o0 + w],
                            start=(cc == 0), stop=(cc == 4))
                vev = stg.tile([128, 640], BF16, tag="vev")
                nc.vector.tensor_copy(vev[:], pv[:])
                nc.sync.dma_start(ve_in.ap()[j * 128:(j + 1) * 128, :],
                                  vev[:])
        nc.gpsimd.collective_compute("AllGather", BYPASS, replica_groups=RG,
                                     ins=[ke_in.ap()[:]],
                                     outs=[ke_out.ap()[:]])
        nc.gpsimd.collective_compute("AllGather", BYPASS, replica_groups=RG,
                                     ins=[ve_in.ap()[:]],
                                     outs=[ve_out.ap()[:]])

        ctx_w.__exit__(None, None, None)   # free hT / ent weights / aug srcs

        # ---- ent pool + early loads of gathered ke/ve (overlap phase C) --
        ctx_e = ExitStack()
        ctx_e.__enter__()
        ep = ctx_e.enter_context(tc.tile_pool(name="entp", bufs=1))
        keT = ep.tile([128, 5 * S], BF16, tag="keT")
        veb = ep.tile([128, 32 * C], BF16, tag="veb")
        wof_sb = ep.tile([128, 5 * C], BF16, tag="wof")
        oTe = ep.tile([128, 5 * SL], BF16, tag="oTe")
        # ================= phase C: heads attention =====================
        groups = [(g * 2, 2) for g in range(16)]
        with tc.tile_pool(name="psp", bufs=3, space="PSUM") as psp, \
             tc.tile_pool(name="pavp", bufs=1, space="PSUM") as pavp, \
             tc.tile_pool(name="pBp", bufs=1, space="PSUM") as pBp, \
             tc.tile_pool(name="ptp", bufs=4) as ptp, \
             tc.tile_pool(name="otp", bufs=2) as otp:
            pend_fin = [None]

            def norm_b():
                if pend_fin[0] is None:
                    return
                ot2, recb2, br2, q02 = pend_fin[0]
                pend_fin[0] = None
                pB = pBp.tile([80, 512], F32, tag="pB")
                nc.tensor.matmul(pB[:], ones80[0:1, 0:80], recb2[:],
                                 start=True, stop=True)
                nc.vector.tensor_mul(hsT[br2][:, q02:q02 + 512], ot2[:],
                                     pB[:])

            for br in ("o", "u"):
                ka, qa, vs = kaug[br], qaug[br], vsb[br]
                for qc in range(8):
                    q0 = qc * 512
                    pav = pavp.tile([97, 512], F32, tag="pav")
                    pend = None

                    def flush(pend):
                        ps, k0g, nk = pend
                        w = nk * 512
                        pt = ptp.tile([128, 1024], BF16, tag="pt")
                        nc.scalar.activation(pt[:, 0:w], ps[:, 0:w], EXP)
                        for j in range(nk):
                            kt = k0g + j
                            nc.tensor.matmul(
                                pav[:], vs[:, kt * 97:kt * 97 + 97],
                                pt[:, j * 512:(j + 1) * 512],
                                start=(kt == 0), stop=(kt == 31))

                    for gi, (k0g, nk) in enumerate(groups):
                        ps = psp.tile([128, 1024], F32, tag="ps")
                        for j in range(nk):
                            kt = k0g + j
                            nc.tensor.matmul(
                                ps[:, j * 512:(j + 1) * 512],
                                ka[:, kt * 128:(kt + 1) * 128],
                                qa[:, q0:q0 + 512], start=True, stop=True)
                        if pend is not None:
                            flush(pend)
                        if gi == 1:
                            norm_b()
                        pend = (ps, k0g, nk)
                    flush(pend)
                    # normalize -> hsT
                    den = otp.tile([1, 512], F32, tag="den")
                    nc.vector.tensor_copy(den[:], pav[96:97, :])
                    rec = otp.tile([1, 512], F32, tag="rec")
                    nc.vector.reciprocal(rec[:], den[:])
                    recb = otp.tile([1, 512], BF16, tag="recb")
                    nc.vector.tensor_copy(recb[:], rec[:])
                    ot = otp.tile([80, 512], F32, tag="ot")
                    nc.vector.tensor_copy(ot[:], pav[0:80, :])
                    pend_fin[0] = (ot, recb, br, q0)
            norm_b()

        for cc in range(5):
            nc.scalar.dma_start(wof_sb[:, cc * C:(cc + 1) * C],
                                d["wof"].ap()[cc * 128:(cc + 1) * 128, :])
        for dc in range(5):
            for r in range(NCORES):
                eng = nc.sync if (r % 2 == 0) else nc.scalar
                eng.dma_start(
                    keT[:, dc * S + r * SL:dc * S + (r + 1) * SL],
                    ke_out.ap()[r * C + dc * 128:r * C + (dc + 1) * 128, :])
        for kt in range(32):
            eng = nc.sync if (kt % 2 == 0) else nc.scalar
            eng.dma_start(veb[:, kt * C:(kt + 1) * C],
                          ve_out.ap()[kt * 128:(kt + 1) * 128, :])

        # ================= phase D: Wo partials + ReduceScatter =========
        with tc.tile_pool(name="wop", bufs=2) as wop, \
             tc.tile_pool(name="wo2_ps", bufs=2, space="PSUM") as wo2ps:
            for st in range(32):
                pw = wo2ps.tile([128, C], F32, tag="pwo")
                for (o0, w) in ((0, 512), (512, 128)):
                    nc.tensor.matmul(pw[:, o0:o0 + w],
                                     hsT["o"][:, st * 128:(st + 1) * 128],
                                     woh_sb[:, o0:o0 + w],
                                     start=True, stop=False)
                    nc.tensor.matmul(pw[:, o0:o0 + w],
                                     hsT["u"][:, st * 128:(st + 1) * 128],
                                     woh_sb[:, o0:o0 + w],
                                     start=False, stop=True)
                pb = wop.tile([128, C], BF16, tag="pbuf")
                nc.vector.tensor_copy(pb[:], pw[:])
                nc.sync.dma_start(p_part.ap()[st * 128:(st + 1) * 128, :],
                                  pb[:])
        nc.gpsimd.collective_compute("ReduceScatter", ADD, replica_groups=RG,
                                     ins=[p_part.ap()[:]],
                                     outs=[p_red.ap()[:]])

        # ================= phase E: ent attention =======================
        with tc.tile_pool(name="psep", bufs=2, space="PSUM") as psep, \
             tc.tile_pool(name="pavep", bufs=1, space="PSUM") as pavep, \
             tc.tile_pool(name="pdenp", bufs=1, space="PSUM") as pdenp, \
             tc.tile_pool(name="ptep", bufs=3) as ptep, \
             tc.tile_pool(name="ote2", bufs=2) as ot2:
            pave = pavep.tile([128, 5 * SL], F32, tag="pave")
            pden = pdenp.tile([1, SL], F32, tag="pden")
            pend = None

            def flush_e(pend):
                ps, kt = pend
                pt = ptep.tile([128, SL], BF16, tag="pte")
                nc.scalar.activation(pt[:], ps[:], EXP)
                for dc in range(5):
                    nc.tensor.matmul(
                        pave[:, dc * SL:(dc + 1) * SL],
                        veb[:, kt * C + dc * 128:kt * C + (dc + 1) * 128],
                        pt[:], start=(kt == 0), stop=(kt == 31))
                nc.tensor.matmul(pden[:], ones128[:], pt[:],
                                 start=(kt == 0), stop=(kt == 31))

            for kt in range(32):
                ps = psep.tile([128, SL], F32, tag="pse")
                for dc in range(5):
                    nc.tensor.matmul(
                        ps[:],
                        keT[:, dc * S + kt * 128:dc * S + (kt + 1) * 128],
                        qeb[:, dc * SL:(dc + 1) * SL],
                        start=(dc == 0), stop=False)
                nc.tensor.matmul(ps[:], augk[:, kt * 128:(kt + 1) * 128],
                                 augqe[:], start=False, stop=True)
                if pend is not None:
                    flush_e(pend)
                pend = (ps, kt)
            flush_e(pend)
            # evict + normalization scale
            for dc in range(5):
                nc.scalar.activation(oTe[:, dc * SL:(dc + 1) * SL],
                                     pave[:, dc * SL:(dc + 1) * SL], COPY)
            den = ot2.tile([1, SL], F32, tag="dene")
            nc.vector.tensor_copy(den[:], pden[:])
            rec = ot2.tile([1, SL], F32, tag="rece")
            nc.vector.reciprocal(rec[:], den[:])
            for st in range(4):
                nc.sync.dma_start(recT[:, st:st + 1],
                                  rec[0:1, st * 128:(st + 1) * 128])
        with tc.tile_pool(name="wo_ps", bufs=2, space="PSUM") as wops:
            for st in range(4):
                pwe = wops.tile([128, C], F32, tag="pwoe")
                for (o0, w) in ((0, 512), (512, 128)):
                    for dc in range(5):
                        nc.tensor.matmul(
                            pwe[:, o0:o0 + w],
                            oTe[:, dc * SL + st * 128:dc * SL + (st + 1) * 128],
                            wof_sb[:, dc * C + o0:dc * C + o0 + w],
                            start=(dc == 0), stop=(dc == 4))
                nc.scalar.activation(entout[:, st * C:(st + 1) * C], pwe[:],
                                     COPY, scale=recT[:, st:st + 1])

        ctx_e.__exit__(None, None, None)

        # ================= phase F: P_red + ent + residual ==============
        with tc.tile_pool(name="finp", bufs=1) as fp:
            pred = fp.tile([128, 4 * C], BF16, tag="pred")
            resid = fp.tile([128, 4 * C], F32, tag="resid")
            for st in range(4):
                nc.sync.dma_start(pred[:, st * C:(st + 1) * C],
                                  p_red.ap()[st * 128:(st + 1) * 128, :])
                nc.scalar.dma_start(resid[:, st * C:(st + 1) * C],
                                    d["res"].ap()[st * 128:(st + 1) * 128, :])
            outt = fp.tile([128, 4 * C], F32, tag="outt")
            nc.vector.tensor_add(outt[:], pred[:], entout[:])
            nc.vector.tensor_add(outt[:], outt[:], resid[:])
            for st in range(4):
                nc.sync.dma_start(out_d.ap()[st * 128:(st + 1) * 128, :],
                                  outt[:, st * C:(st + 1) * C])


def build_in_maps(hidden_states, mask, inpainting_mask, Wq, Wk, Wv,
                  Wq_ent, Wk_ent, Wv_ent, Wq_out, Wk_out, Wv_out, Wo):
    BF = ml_dtypes.bfloat16
    h = np.asarray(hidden_states[0], np.float32)          # [S, C]
    hT = np.ascontiguousarray(h.T)                         # [C, S]
    m2 = np.asarray(mask[0, 0], np.int32)
    im2 = np.asarray(inpainting_mask[0, 0], np.int32)
    mrow = np.ascontiguousarray(m2[::8, ::8]).reshape(1, S)
    imrow = np.ascontiguousarray(im2[::8, ::8]).reshape(1, S)
    WoT = np.ascontiguousarray(np.asarray(Wo, np.float32).T)  # [C, C]
    sq = np.float32(1.0 / np.sqrt(80.0))
    se = np.float32(1.0 / np.sqrt(640.0))

    def wT(W):
        return np.ascontiguousarray(np.asarray(W, np.float32).T)

    wqe_b = wT(np.asarray(Wq_ent) * se).astype(BF)
    wke_b = wT(Wk_ent).astype(BF)
    wve_b = wT(Wv_ent).astype(BF)
    wof_b = WoT.astype(BF)
    hT_b = hT.astype(BF)

    in_maps = []
    for i in range(NCORES):
        hd = slice(D * i, D * (i + 1))
        ql = slice(SL * i, SL * (i + 1))
        in_maps.append({
            "hT": hT_b,
            "hq": np.ascontiguousarray(hT_b[:, ql]),
            "res": np.ascontiguousarray(h[ql, :]),
            "wq": wT(np.asarray(Wq)[hd] * sq).astype(BF),
            "wk": wT(np.asarray(Wk)[hd]).astype(BF),
            "wv": wT(np.asarray(Wv)[hd]).astype(BF),
            "wqo": wT(np.asarray(Wq_out)[hd] * sq).astype(BF),
            "wko": wT(np.asarray(Wk_out)[hd]).astype(BF),
            "wvo": wT(np.asarray(Wv_out)[hd]).astype(BF),
            "wqe": wqe_b, "wke": wke_b, "wve": wve_b, "wof": wof_b,
            "woh": np.ascontiguousarray(WoT[hd, :]).astype(BF),
            "mrow": mrow, "imrow": imrow,
            "mq": np.ascontiguousarray(mrow[:, ql]),
        })
    return in_maps


def kernel(**inputs):
    in_maps = build_in_maps(**inputs)
    if "nc" not in _cache:
        _cache["nc"] = _build()
    res = run_bass_kernel_spmd(_cache["nc"], in_maps, list(range(NCORES)),
                               trace=False)
    out = np.concatenate([res.results[i]["out"] for i in range(NCORES)], axis=0)
    return out.reshape(1, S, C).astype(np.float32)
